# revision 20
# baseline (speedup 1.0000x reference)
"""GCN (3-layer GCNConv + BN/ReLU + global mean pool + sigmoid) on 8 trn2
NeuronCores via Bass/Tile.

Strategy (per sharding hint): 1D-partition the 100K nodes across 8 cores
(12500 each).  Edges (incl. self-loops) are bucketed by destination core /
128-node destination window / 25000-row source chunk on the host.  Each
layer: aggregate-first formulation  conv = diag(dinv) @ A_raw @ (diag(dinv)
@ h) @ W  computed as
  - dma_gather of scaled source rows h'[src] from a replicated (AllGather'd)
    node-major table in HBM; the 4 source chunks are issued on 4 SWDGE
    queues so descriptor generation runs on 4 Q7 cpu pairs in parallel,
  - segment-sum via TensorE matmuls against one-hot indicator matrices built
    on VectorE with a broadcast is_equal against an iota row,
  - per-dst dinv scaling (rank-1 PE broadcast of the dinv row),
  - dense GEMM with the replicated [d,d] weight,
  - BatchNorm with global stats via a tiny AllReduce, fused ReLU on ScalarE.
Graph mean-pool = indicator matmul against one-hot graph ids + AllReduce.
"""
import sys
sys.path.insert(0, "/opt/trn_rl_repo")

import numpy as np

N = 100000
E = 1600000
NCORES = 8
NLOC = N // NCORES          # 12500 nodes per core
D = 128
DOUT = 32
G = 64
NW = (NLOC + 127) // 128    # 98 windows (last has 84 nodes)
NWP = NW * 128              # 12544 padded local node slots
CH = 25000                  # source chunk rows (int16-indexable)
NCH = 4
SBW = 3                     # windows per superblock
EPS = 1e-5


def _ceil128(x):
    return (np.asarray(x) + 127) // 128 * 128


def _prep(x, edge_index, batch):
    """Host-side graph partitioning. Returns (layout, per_core_arrays)."""
    src0 = np.asarray(edge_index[0], dtype=np.int64)
    dst0 = np.asarray(edge_index[1], dtype=np.int64)
    loop = np.arange(N, dtype=np.int64)
    srcs = np.concatenate([src0, loop])
    dsts = np.concatenate([dst0, loop])

    deg = np.bincount(dsts, minlength=N).astype(np.float64)
    dinv = (1.0 / np.sqrt(np.maximum(deg, 1.0))).astype(np.float32)
    dinv[deg == 0] = 0.0

    core = dsts // NLOC
    nloc = dsts % NLOC
    win = nloc >> 7
    dl = (nloc & 127).astype(np.float32)
    ch = srcs // CH
    il = (srcs % CH).astype(np.int16)

    key = ((core * NW + win) * NCH + ch).astype(np.int64)
    order = np.argsort(key, kind="stable")
    il_s = il[order]
    dl_s = dl[order]
    cnts = np.bincount(key, minlength=NCORES * NW * NCH).reshape(
        NCORES, NW, NCH)
    starts = np.zeros(NCORES * NW * NCH + 1, np.int64)
    np.cumsum(cnts.ravel(), out=starts[1:])

    pad = _ceil128(cnts.max(axis=0)).astype(np.int64)   # [NW, NCH]
    nblk_wc = pad // 128                                # [NW, NCH]
    nblk_w = nblk_wc.sum(axis=1)                        # [NW]
    cblk = np.zeros((NW, NCH), np.int64)                # block off within win
    cblk[:, 1:] = np.cumsum(nblk_wc[:, :-1], axis=1)
    blkoff = np.zeros(NW + 1, np.int64)                 # global dstloc col off
    np.cumsum(nblk_w, out=blkoff[1:])
    nblk_tot = int(blkoff[-1])

    sbs = [list(range(i, min(i + SBW, NW))) for i in range(0, NW, SBW)]
    # idx16 column layout: per sb, per chunk call
    o16 = {}
    col16 = 0
    for sbi, ws in enumerate(sbs):
        for c in range(NCH):
            L = int(pad[ws, c].sum())
            o16[(sbi, c)] = (col16, L)
            col16 += L // 16
    cols16_tot = col16

    layout = dict(pad=pad, nblk_wc=nblk_wc, nblk_w=nblk_w, cblk=cblk,
                  blkoff=blkoff, nblk_tot=nblk_tot, sbs=sbs, o16=o16,
                  cols16_tot=cols16_tot)

    per_core = []
    batch = np.asarray(batch, dtype=np.int64)
    cnt_g = np.bincount(batch, minlength=G).astype(np.float32)
    cnt_inv = (1.0 / np.maximum(cnt_g, 1.0)).reshape(G, 1).astype(np.float32)

    for r in range(NCORES):
        idx16 = np.zeros((16, cols16_tot), np.int16)
        dstloc = np.full((128, nblk_tot), -1.0, np.float32)
        for sbi, ws in enumerate(sbs):
            for c in range(NCH):
                c0, L = o16[(sbi, c)]
                if L == 0:
                    continue
                flat = np.zeros(L, np.int16)
                q0 = 0
                for w in ws:
                    gk = (r * NW + w) * NCH + c
                    s = int(starts[gk])
                    n = int(cnts[r, w, c])
                    if n:
                        flat[q0:q0 + n] = il_s[s:s + n]
                        t = np.arange(n)
                        dstloc[t & 127,
                               blkoff[w] + cblk[w, c] + (t >> 7)] = \
                            dl_s[s:s + n]
                    q0 += int(pad[w, c])
                idx16[:, c0:c0 + L // 16] = flat.reshape(L // 16, 16).T
        idx16 = np.tile(idx16, (8, 1))  # [128, cols16_tot]

        nds = np.arange(NWP)
        gl = r * NLOC + nds
        valid = nds < NLOC
        dv = np.where(valid, dinv[np.minimum(gl, N - 1)], 0.0).astype(
            np.float32)
        dinv_local = dv.reshape(NW, 128).T.copy()          # [128, NW]
        dinv_row = dv.reshape(1, NWP).copy()               # [1, NWP]
        bl = np.where(valid, batch[np.minimum(gl, N - 1)], -1.0).astype(
            np.float32)
        batch_local = bl.reshape(NW, 128).T.copy()         # [128, NW]
        x_local = np.asarray(x[r * NLOC:(r + 1) * NLOC], np.float32)

        per_core.append(dict(idx16=idx16, dstloc=dstloc,
                             dinv_local=dinv_local, dinv_row=dinv_row,
                             batch_local=batch_local, x_local=x_local,
                             cnt_inv=cnt_inv))
    return layout, per_core


def _build(layout):
    import os
    import concourse.tile as tile
    from concourse import bacc, mybir

    ph = int(os.environ.get("KPHASE", "99"))
    nocc = bool(int(os.environ.get("KNOCC", "0")))
    kskip = os.environ.get("KSKIP", "")

    f32 = mybir.dt.float32
    DT = f32
    pad = layout["pad"]
    nblk_wc = layout["nblk_wc"]
    nblk_w = layout["nblk_w"]
    cblk = layout["cblk"]
    blkoff = layout["blkoff"]
    nblk_tot = layout["nblk_tot"]
    sbs = layout["sbs"]
    o16 = layout["o16"]
    cols16_tot = layout["cols16_tot"]

    nc = bacc.Bacc("TRN2", target_bir_lowering=False, debug=False,
                   num_devices=NCORES, num_swdge_queues=4)

    def din(name, shape, dt=f32):
        return nc.dram_tensor(name, shape, dt, kind="ExternalInput")

    x_local = din("x_local", [NLOC, D])
    idx16 = din("idx16", [128, cols16_tot], mybir.dt.int16)
    dstloc_d = din("dstloc", [128, nblk_tot])
    dinv_local_d = din("dinv_local", [128, NW])
    dinv_row_d = din("dinv_row", [1, NWP])
    batch_local_d = din("batch_local", [128, NW])
    cnt_inv_d = din("cnt_inv", [G, 1])
    iota_d = din("iota", [128, D])
    ident_d = din("ident", [128, D])
    W_d = [din("W1", [D, D]), din("W2", [D, D]), din("W3", [D, DOUT])]
    b3_d = din("b3", [DOUT, 1])
    gam_d = [din("gamma1", [D, 1]), din("gamma2", [D, 1])]
    bet_d = [din("beta1", [D, 1]), din("beta2", [D, 1])]
    out_d = nc.dram_tensor("out", [G, DOUT], f32, kind="ExternalOutput")

    from contextlib import ExitStack
    with tile.TileContext(nc) as tc, ExitStack() as _ctx:
        ec = _ctx.enter_context
        cp = ec(tc.tile_pool(name="const", bufs=1))
        convp = ec(tc.tile_pool(name="conv", bufs=1))
        xpp = ec(tc.tile_pool(name="xprep", bufs=4))
        idxp = ec(tc.tile_pool(name="idxs", bufs=2))
        dlp = ec(tc.tile_pool(name="dls", bufs=2))
        msgp = ec(tc.tile_pool(name="msg", bufs=6))
        Sp = ec(tc.tile_pool(name="Sp", bufs=2))
        aggp = ec(tc.tile_pool(name="agg", bufs=2))
        dvp = ec(tc.tile_pool(name="dv", bufs=2))
        smlp = ec(tc.tile_pool(name="sml", bufs=16))
        sqp = ec(tc.tile_pool(name="sq", bufs=2))
        gwp = ec(tc.tile_pool(name="gw", bufs=4))
        dramp = ec(tc.tile_pool(name="dram", bufs=1, space="DRAM"))
        # PSUM bank budget (8 banks): win/tp/t3 share 4 rotating slots,
        # dv 1, gemm 2, pooled 1.
        psW = ec(tc.tile_pool(name="psW", bufs=3, space="PSUM"))
        psG = ec(tc.tile_pool(name="psG", bufs=2, space="PSUM"))
        psP = ec(tc.tile_pool(name="psP", bufs=1, space="PSUM"))
        if True:
            # ---- constants ----
            iota_t = cp.tile([128, D], f32, tag="iota")
            nc.sync.dma_start(iota_t[:], iota_d[:])
            id_t = cp.tile([128, D], f32, tag="ident")
            nc.sync.dma_start(id_t[:], ident_d[:])
            dvl_t = cp.tile([128, NW], f32, tag="dvl")
            nc.sync.dma_start(dvl_t[:], dinv_local_d[:])
            bat_t = cp.tile([128, NW], f32, tag="bat")
            nc.sync.dma_start(bat_t[:], batch_local_d[:])
            ci_t = cp.tile([G, 1], f32, tag="ci")
            nc.sync.dma_start(ci_t[:], cnt_inv_d[:])
            W_t = []
            for li in range(3):
                fo = DOUT if li == 2 else D
                wt = cp.tile([D, fo], f32, tag=f"W{li}", name=f"Wt{li}")
                nc.sync.dma_start(wt[:], W_d[li][:])
                W_t.append(wt)
            b3_t = cp.tile([DOUT, 1], f32, tag="b3")
            nc.sync.dma_start(b3_t[:], b3_d[:])
            gam_t, bet_t = [], []
            for li in range(2):
                g = cp.tile([D, 1], f32, tag=f"g{li}", name=f"gam{li}")
                nc.sync.dma_start(g[:], gam_d[li][:])
                gam_t.append(g)
                b = cp.tile([D, 1], f32, tag=f"be{li}", name=f"bet{li}")
                nc.sync.dma_start(b[:], bet_d[li][:])
                bet_t.append(b)

            # ---- DRAM internals ----
            table = dramp.tile([N, D], DT, tag="table")
            agin = dramp.tile([NLOC, D], DT, tag="agin")
            ar_b = [(dramp.tile([128, 2], f32, tag=f"ari{i}",
                                name=f"ari{i}"),
                     dramp.tile([128, 2], f32, tag=f"aro{i}",
                                name=f"aro{i}"))
                    for i in range(2)]
            arp_i = dramp.tile([G, DOUT], f32, tag="arpi")
            arp_o = dramp.tile([G, DOUT], f32, tag="arpo")

            rg = [list(range(NCORES))]

            def prep_table(src_feat_major=None):
                """Write dinv-scaled node-major rows into agin, AllGather."""
                for w in range(NW):
                    cnt = min(128, NLOC - w * 128)
                    if src_feat_major is None:
                        xt = xpp.tile([128, D], f32, tag="xt")
                        nc.sync.dma_start(xt[:cnt, :],
                                          x_local[w * 128:w * 128 + cnt, :])
                        src_nm = xt
                    else:
                        tps = psW.tile([128, D], f32, tag="win",
                                       space="PSUM")
                        nc.tensor.transpose(
                            tps[:], src_feat_major[:D, w * 128:(w + 1) * 128],
                            id_t[:])
                        src_nm = tps
                    hq = xpp.tile([128, D], DT, tag="hq")
                    nc.scalar.activation(
                        hq[:cnt, :], src_nm[:cnt, :],
                        mybir.ActivationFunctionType.Copy,
                        bias=0.0, scale=dvl_t[:cnt, w:w + 1])
                    nc.sync.dma_start(agin[w * 128:w * 128 + cnt, :],
                                      hq[:cnt, :])
                if not nocc:
                    nc.gpsimd.collective_compute(
                        "AllGather", mybir.AluOpType.bypass,
                        replica_groups=rg, ins=[agin.opt()],
                        outs=[table.opt()])

            prep_table(None)

            conv = None
            _lireq = {0: 1, 1: 3, 2: 4}
            for li in range(3):
                if ph < _lireq[li]:
                    break
                fo = DOUT if li == 2 else D
                conv = convp.tile([128, NWP], f32, tag="conv")
                for sbi, ws in enumerate(sbs):
                    ncols = len(ws) * 128
                    w0 = ws[0]
                    # stream idx + dstloc for this superblock
                    c16_0 = o16[(sbi, 0)][0]
                    c16_end = o16[(sbi, NCH - 1)][0] + \
                        o16[(sbi, NCH - 1)][1] // 16
                    idxt = idxp.tile([128, c16_end - c16_0], mybir.dt.int16,
                                     tag="idxt")
                    nc.sync.dma_start(idxt[:], idx16[:, c16_0:c16_end])
                    nb0 = int(blkoff[w0])
                    nb_sb = int(blkoff[ws[-1] + 1] - nb0)
                    dlt = dlp.tile([128, nb_sb], f32, tag="dlt")
                    nc.sync.dma_start(dlt[:], dstloc_d[:, nb0:nb0 + nb_sb])

                    msgs = []
                    for c in range(NCH):
                        c0, L = o16[(sbi, c)]
                        nblk_sc = L // 128
                        mt = msgp.tile([128, max(nblk_sc, 1), D], DT,
                                       tag="mt")
                        if L and "gather" not in kskip:
                            nc.gpsimd.dma_gather(
                                mt[:, :nblk_sc, :],
                                table[c * CH:(c + 1) * CH, :],
                                idxt[:, c0 - c16_0:c0 - c16_0 + L // 16],
                                L, L, D, single_packet=False,
                                queue_num=c)
                        msgs.append(mt)

                    # dinv_dst broadcast [128, ncols]: partition-bcast DMA
                    dvsb = dvp.tile([128, ncols], f32, tag="dvsb")
                    nc.sync.dma_start(
                        dvsb[:],
                        dinv_row_d[0:1, w0 * 128:w0 * 128 + ncols]
                        .to_broadcast([128, ncols]))

                    aggT = aggp.tile([128, ncols], f32, tag="aggT")
                    for wi, w in enumerate(ws):
                        nbw = int(nblk_w[w])
                        if nbw == 0:
                            nc.vector.memset(aggT[:, wi * 128:(wi + 1) * 128],
                                             0.0)
                            continue
                        if "mm" in kskip:
                            nc.vector.memset(
                                aggT[:, wi * 128:(wi + 1) * 128], 0.0)
                            continue
                        rel = int(blkoff[w]) - nb0
                        Sw = Sp.tile([128, nbw, D], DT, tag="Sw")
                        nc.vector.tensor_tensor(
                            out=Sw[:],
                            in0=iota_t[:].rearrange("p (n f) -> p n f", n=1)
                                         .to_broadcast([128, nbw, D]),
                            in1=dlt[:, rel:rel + nbw]
                                .rearrange("p (n f) -> p n f", f=1)
                                .to_broadcast([128, nbw, D]),
                            op=mybir.AluOpType.is_equal)
                        ps = psW.tile([128, D], f32, tag="win", space="PSUM")
                        ops = []
                        for c in range(NCH):
                            bco = int(pad[[w2 for w2 in ws if w2 < w], c]
                                      .sum()) // 128 if ws else 0
                            for j in range(int(nblk_wc[w, c])):
                                ops.append((c, bco + j,
                                            int(cblk[w, c]) + j))
                        for k, (c, b, scol) in enumerate(ops):
                            nc.tensor.matmul(
                                ps[:], lhsT=msgs[c][:, b, :],
                                rhs=Sw[:, scol, :],
                                start=(k == 0), stop=(k == len(ops) - 1))
                        nc.vector.tensor_tensor(
                            out=aggT[:, wi * 128:(wi + 1) * 128],
                            in0=ps[:], in1=dvsb[:, wi * 128:(wi + 1) * 128],
                            op=mybir.AluOpType.mult)

                    gps = psG.tile([fo, ncols], f32, tag="gps", space="PSUM")
                    nc.tensor.matmul(gps[:], lhsT=W_t[li][:, :fo],
                                     rhs=aggT[:, :ncols],
                                     start=True, stop=True)
                    cc = w0 * 128
                    if li < 2:
                        nc.scalar.copy(conv[:fo, cc:cc + ncols], gps[:])
                    else:
                        nc.scalar.activation(
                            conv[:fo, cc:cc + ncols], gps[:],
                            mybir.ActivationFunctionType.Identity,
                            bias=b3_t[:, 0:1], scale=1.0)

                if li == 0 and ph < 2:
                    break
                if li < 2:
                    # ---- BatchNorm (global stats) + ReLU ----
                    stats = smlp.tile([128, 2], f32, tag="stats")
                    nc.vector.tensor_reduce(stats[:, 0:1], conv[:D, :NWP],
                                            mybir.AxisListType.X,
                                            mybir.AluOpType.add)
                    nchunk = (NWP + 511) // 512
                    sqcols = smlp.tile([128, nchunk], f32, tag="sqcols")
                    for k in range(nchunk):
                        a = k * 512
                        b = min(NWP, a + 512)
                        sq = sqp.tile([128, 512], f32, tag="sq")
                        nc.scalar.square(sq[:, :b - a], conv[:D, a:b])
                        nc.vector.tensor_reduce(
                            sqcols[:, k:k + 1], sq[:, :b - a],
                            mybir.AxisListType.X, mybir.AluOpType.add)
                    nc.vector.tensor_reduce(stats[:, 1:2], sqcols[:],
                                            mybir.AxisListType.X,
                                            mybir.AluOpType.add)
                    if ph == 20:
                        break
                    ari, aro = ar_b[li]
                    nc.sync.dma_start(ari[:], stats[:])
                    if not nocc:
                        nc.gpsimd.collective_compute(
                            "AllReduce", mybir.AluOpType.add,
                            replica_groups=rg, ins=[ari.opt()],
                            outs=[aro.opt()])
                    sg = smlp.tile([128, 2], f32, tag="sg")
                    nc.sync.dma_start(sg[:], aro[:])
                    if ph == 21:
                        break
                    mean = smlp.tile([128, 1], f32, tag="mean")
                    nc.vector.tensor_scalar(mean[:], sg[:, 0:1], 1.0 / N,
                                            None, mybir.AluOpType.mult)
                    ex2 = smlp.tile([128, 1], f32, tag="ex2")
                    nc.vector.tensor_scalar(ex2[:], sg[:, 1:2], 1.0 / N,
                                            None, mybir.AluOpType.mult)
                    var = smlp.tile([128, 1], f32, tag="var")
                    nc.vector.tensor_tensor(var[:], mean[:], mean[:],
                                            op=mybir.AluOpType.mult)
                    nc.vector.tensor_tensor(var[:], ex2[:], var[:],
                                            op=mybir.AluOpType.subtract)
                    nc.vector.tensor_scalar(var[:], var[:], EPS, None,
                                            mybir.AluOpType.add)
                    std = smlp.tile([128, 1], f32, tag="std")
                    nc.scalar.sqrt(std[:], var[:])
                    istd = smlp.tile([128, 1], f32, tag="istd")
                    nc.vector.reciprocal(istd[:], std[:])
                    sco = smlp.tile([128, 1], f32, tag="sco")
                    nc.vector.tensor_tensor(sco[:], gam_t[li][:], istd[:],
                                            op=mybir.AluOpType.mult)
                    sh = smlp.tile([128, 1], f32, tag="sh")
                    nc.vector.tensor_tensor(sh[:], mean[:], sco[:],
                                            op=mybir.AluOpType.mult)
                    nc.vector.tensor_tensor(sh[:], bet_t[li][:], sh[:],
                                            op=mybir.AluOpType.subtract)
                    nc.scalar.activation(conv[:D, :NWP], conv[:D, :NWP],
                                         mybir.ActivationFunctionType.Relu,
                                         bias=sh[:, 0:1], scale=sco[:, 0:1])
                    if ph == 22:
                        break
                    prep_table(conv)
                elif ph >= 5:
                    # ---- global mean pool + sigmoid ----
                    pooled = psP.tile([G, DOUT], f32, tag="pooled",
                                      space="PSUM")
                    for w in range(NW):
                        Gw = gwp.tile([128, G], f32, tag="Gw")
                        nc.vector.tensor_tensor(
                            out=Gw[:], in0=iota_t[:, :G],
                            in1=bat_t[:, w:w + 1].to_broadcast([128, G]),
                            op=mybir.AluOpType.is_equal)
                        t3 = psW.tile([128, D], f32, tag="win",
                                      space="PSUM")
                        nc.tensor.transpose(
                            t3[:, :DOUT], conv[:DOUT, w * 128:(w + 1) * 128],
                            id_t[:DOUT, :DOUT])
                        c3 = gwp.tile([128, DOUT], f32, tag="c3")
                        nc.scalar.copy(c3[:], t3[:, :DOUT])
                        nc.tensor.matmul(pooled[:], lhsT=Gw[:], rhs=c3[:],
                                         start=(w == 0), stop=(w == NW - 1))
                    psb = smlp.tile([G, DOUT], f32, tag="psb")
                    nc.scalar.copy(psb[:], pooled[:])
                    nc.sync.dma_start(arp_i[:], psb[:])
                    if not nocc:
                        nc.gpsimd.collective_compute(
                            "AllReduce", mybir.AluOpType.add,
                            replica_groups=rg, ins=[arp_i.opt()],
                            outs=[arp_o.opt()])
                    pall = smlp.tile([G, DOUT], f32, tag="pall")
                    nc.sync.dma_start(pall[:], arp_o[:])
                    fin = smlp.tile([G, DOUT], f32, tag="fin")
                    nc.scalar.activation(
                        fin[:], pall[:],
                        mybir.ActivationFunctionType.Sigmoid,
                        bias=0.0, scale=ci_t[:, 0:1])
                    nc.sync.dma_start(out_d[:], fin[:])

    nc.compile()
    return nc


def prepare(x, edge_index, batch, W1, b1, W2, b2, W3, b3,
            gamma1, beta1, gamma2, beta2):
    """Build the Bass program + per-core input maps."""
    layout, per_core = _prep(np.asarray(x, np.float32), edge_index, batch)
    nc = _build(layout)

    iota = np.broadcast_to(np.arange(D, dtype=np.float32), (128, D)).copy()
    ident = np.eye(D, dtype=np.float32)
    shared = {
        "iota": iota, "ident": ident,
        "W1": np.asarray(W1, np.float32), "W2": np.asarray(W2, np.float32),
        "W3": np.asarray(W3, np.float32),
        "b3": np.asarray(b3, np.float32).reshape(DOUT, 1),
        "gamma1": np.asarray(gamma1, np.float32).reshape(D, 1),
        "gamma2": np.asarray(gamma2, np.float32).reshape(D, 1),
        "beta1": np.asarray(beta1, np.float32).reshape(D, 1),
        "beta2": np.asarray(beta2, np.float32).reshape(D, 1),
    }
    in_maps = []
    for r in range(NCORES):
        pc = per_core[r]
        in_maps.append({
            "x_local": pc["x_local"], "idx16": pc["idx16"],
            "dstloc": pc["dstloc"], "dinv_local": pc["dinv_local"],
            "dinv_row": pc["dinv_row"], "batch_local": pc["batch_local"],
            "cnt_inv": pc["cnt_inv"], **shared,
        })

    return nc, in_maps


def run_on_hw(nc, in_maps):
    from concourse.bass_utils import run_bass_kernel_spmd
    last = None
    for attempt in range(3):
        try:
            res = run_bass_kernel_spmd(nc, in_maps,
                                       core_ids=list(range(NCORES)))
            return np.asarray(res.results[0]["out"], np.float32)
        except Exception as e:  # transient device wedges happen
            last = e
    raise last


def kernel(x, edge_index, batch, W1, b1, W2, b2, W3, b3,
           gamma1, beta1, gamma2, beta2):
    nc, in_maps = prepare(x, edge_index, batch, W1, b1, W2, b2, W3, b3,
                          gamma1, beta1, gamma2, beta2)
    return run_on_hw(nc, in_maps)


if __name__ == "__main__":
    sys.path.insert(0, "/root/problem")
    import reference
    inputs = {k: np.asarray(v) for k, v in reference.setup_inputs().items()}
    out = kernel(**inputs)
    print("out", out.shape, out.dtype)


# revision 21
# speedup vs baseline: 1.3445x; 1.3445x over previous
"""GCN (3-layer GCNConv + BN/ReLU + global mean pool + sigmoid) on 8 trn2
NeuronCores via Bass/Tile.

Strategy (per sharding hint): 1D-partition the 100K nodes across 8 cores
(12500 each).  Edges (incl. self-loops) are bucketed by destination core /
128-node destination window / 25000-row source chunk on the host.  Each
layer: aggregate-first formulation  conv = diag(dinv) @ A_raw @ (diag(dinv)
@ h) @ W  computed as
  - dma_gather of scaled source rows h'[src] from a replicated (AllGather'd)
    node-major table in HBM; the 4 source chunks are issued on 4 SWDGE
    queues so descriptor generation runs on 4 Q7 cpu pairs in parallel,
  - segment-sum via TensorE matmuls against one-hot indicator matrices built
    on VectorE with a broadcast is_equal against an iota row,
  - per-dst dinv scaling (rank-1 PE broadcast of the dinv row),
  - dense GEMM with the replicated [d,d] weight,
  - BatchNorm with global stats via a tiny AllReduce, fused ReLU on ScalarE.
Graph mean-pool = indicator matmul against one-hot graph ids + AllReduce.
"""
import sys
sys.path.insert(0, "/opt/trn_rl_repo")

import numpy as np

N = 100000
E = 1600000
NCORES = 8
NLOC = N // NCORES          # 12500 nodes per core
D = 128
DOUT = 32
G = 64
NW = (NLOC + 127) // 128    # 98 windows (last has 84 nodes)
NWP = NW * 128              # 12544 padded local node slots
CH = 25000                  # source chunk rows (int16-indexable)
NCH = 4
SBW = 3                     # windows per superblock
EPS = 1e-5


def _ceil128(x):
    return (np.asarray(x) + 127) // 128 * 128


def _prep(x, edge_index, batch):
    """Host-side graph partitioning. Returns (layout, per_core_arrays)."""
    src0 = np.asarray(edge_index[0], dtype=np.int64)
    dst0 = np.asarray(edge_index[1], dtype=np.int64)
    loop = np.arange(N, dtype=np.int64)
    srcs = np.concatenate([src0, loop])
    dsts = np.concatenate([dst0, loop])

    deg = np.bincount(dsts, minlength=N).astype(np.float64)
    dinv = (1.0 / np.sqrt(np.maximum(deg, 1.0))).astype(np.float32)
    dinv[deg == 0] = 0.0

    core = dsts // NLOC
    nloc = dsts % NLOC
    win = nloc >> 7
    dl = (nloc & 127).astype(np.float32)
    ch = srcs // CH
    il = (srcs % CH).astype(np.int16)

    key = ((core * NW + win) * NCH + ch).astype(np.int64)
    order = np.argsort(key, kind="stable")
    il_s = il[order]
    dl_s = dl[order]
    cnts = np.bincount(key, minlength=NCORES * NW * NCH).reshape(
        NCORES, NW, NCH)
    starts = np.zeros(NCORES * NW * NCH + 1, np.int64)
    np.cumsum(cnts.ravel(), out=starts[1:])

    pad = _ceil128(cnts.max(axis=0)).astype(np.int64)   # [NW, NCH]
    nblk_wc = pad // 128                                # [NW, NCH]
    nblk_w = nblk_wc.sum(axis=1)                        # [NW]
    cblk = np.zeros((NW, NCH), np.int64)                # block off within win
    cblk[:, 1:] = np.cumsum(nblk_wc[:, :-1], axis=1)
    blkoff = np.zeros(NW + 1, np.int64)                 # global dstloc col off
    np.cumsum(nblk_w, out=blkoff[1:])
    nblk_tot = int(blkoff[-1])

    sbs = [list(range(i, min(i + SBW, NW))) for i in range(0, NW, SBW)]
    # idx16 column layout: per sb, per chunk call
    o16 = {}
    col16 = 0
    for sbi, ws in enumerate(sbs):
        for c in range(NCH):
            L = int(pad[ws, c].sum())
            o16[(sbi, c)] = (col16, L)
            col16 += L // 16
    cols16_tot = col16

    layout = dict(pad=pad, nblk_wc=nblk_wc, nblk_w=nblk_w, cblk=cblk,
                  blkoff=blkoff, nblk_tot=nblk_tot, sbs=sbs, o16=o16,
                  cols16_tot=cols16_tot)

    per_core = []
    batch = np.asarray(batch, dtype=np.int64)
    cnt_g = np.bincount(batch, minlength=G).astype(np.float32)
    cnt_inv = (1.0 / np.maximum(cnt_g, 1.0)).reshape(G, 1).astype(np.float32)

    for r in range(NCORES):
        idx16 = np.zeros((16, cols16_tot), np.int16)
        dstloc = np.full((128, nblk_tot), -1.0, np.float32)
        for sbi, ws in enumerate(sbs):
            for c in range(NCH):
                c0, L = o16[(sbi, c)]
                if L == 0:
                    continue
                flat = np.zeros(L, np.int16)
                q0 = 0
                for w in ws:
                    gk = (r * NW + w) * NCH + c
                    s = int(starts[gk])
                    n = int(cnts[r, w, c])
                    if n:
                        flat[q0:q0 + n] = il_s[s:s + n]
                        t = np.arange(n)
                        dstloc[t & 127,
                               blkoff[w] + cblk[w, c] + (t >> 7)] = \
                            dl_s[s:s + n]
                    q0 += int(pad[w, c])
                idx16[:, c0:c0 + L // 16] = flat.reshape(L // 16, 16).T
        idx16 = np.tile(idx16, (8, 1))  # [128, cols16_tot]

        nds = np.arange(NWP)
        gl = r * NLOC + nds
        valid = nds < NLOC
        dv = np.where(valid, dinv[np.minimum(gl, N - 1)], 0.0).astype(
            np.float32)
        dinv_local = dv.reshape(NW, 128).T.copy()          # [128, NW]
        dinv_row = dv.reshape(1, NWP).copy()               # [1, NWP]
        bl = np.where(valid, batch[np.minimum(gl, N - 1)], -1.0).astype(
            np.float32)
        batch_local = bl.reshape(NW, 128).T.copy()         # [128, NW]
        x_local = np.asarray(x[r * NLOC:(r + 1) * NLOC], np.float32)

        per_core.append(dict(idx16=idx16, dstloc=dstloc,
                             dinv_local=dinv_local, dinv_row=dinv_row,
                             batch_local=batch_local, x_local=x_local,
                             cnt_inv=cnt_inv))
    return layout, per_core


def _build(layout):
    import os
    import concourse.tile as tile
    from concourse import bacc, mybir

    ph = int(os.environ.get("KPHASE", "99"))
    nocc = bool(int(os.environ.get("KNOCC", "0")))
    kskip = os.environ.get("KSKIP", "")

    f32 = mybir.dt.float32
    DT = f32
    pad = layout["pad"]
    nblk_wc = layout["nblk_wc"]
    nblk_w = layout["nblk_w"]
    cblk = layout["cblk"]
    blkoff = layout["blkoff"]
    nblk_tot = layout["nblk_tot"]
    sbs = layout["sbs"]
    o16 = layout["o16"]
    cols16_tot = layout["cols16_tot"]

    nc = bacc.Bacc("TRN2", target_bir_lowering=False, debug=False,
                   num_devices=NCORES, num_swdge_queues=4)

    def din(name, shape, dt=f32):
        return nc.dram_tensor(name, shape, dt, kind="ExternalInput")

    x_local = din("x_local", [NLOC, D])
    idx16 = din("idx16", [128, cols16_tot], mybir.dt.int16)
    dstloc_d = din("dstloc", [128, nblk_tot])
    dinv_local_d = din("dinv_local", [128, NW])
    dinv_row_d = din("dinv_row", [1, NWP])
    batch_local_d = din("batch_local", [128, NW])
    cnt_inv_d = din("cnt_inv", [G, 1])
    iota_d = din("iota", [128, D])
    ident_d = din("ident", [128, D])
    W_d = [din("W1", [D, D]), din("W2", [D, D]), din("W3", [D, DOUT])]
    b3_d = din("b3", [DOUT, 1])
    gam_d = [din("gamma1", [D, 1]), din("gamma2", [D, 1])]
    bet_d = [din("beta1", [D, 1]), din("beta2", [D, 1])]
    out_d = nc.dram_tensor("out", [G, DOUT], f32, kind="ExternalOutput")

    from contextlib import ExitStack
    with tile.TileContext(nc) as tc, ExitStack() as _ctx:
        ec = _ctx.enter_context
        cp = ec(tc.tile_pool(name="const", bufs=1))
        convp = ec(tc.tile_pool(name="conv", bufs=1))
        xpp = ec(tc.tile_pool(name="xprep", bufs=4))
        idxp = ec(tc.tile_pool(name="idxs", bufs=3))
        dlp = ec(tc.tile_pool(name="dls", bufs=3))
        msgp = ec(tc.tile_pool(name="msg", bufs=8))
        Sp = ec(tc.tile_pool(name="Sp", bufs=2))
        aggp = ec(tc.tile_pool(name="agg", bufs=2))
        dvp = ec(tc.tile_pool(name="dv", bufs=2))
        smlp = ec(tc.tile_pool(name="sml", bufs=16))
        sqp = ec(tc.tile_pool(name="sq", bufs=2))
        gwp = ec(tc.tile_pool(name="gw", bufs=4))
        dramp = ec(tc.tile_pool(name="dram", bufs=1, space="DRAM"))
        # PSUM bank budget (8 banks): win/tp/t3 share 4 rotating slots,
        # dv 1, gemm 2, pooled 1.
        psW = ec(tc.tile_pool(name="psW", bufs=3, space="PSUM"))
        psG = ec(tc.tile_pool(name="psG", bufs=2, space="PSUM"))
        psP = ec(tc.tile_pool(name="psP", bufs=1, space="PSUM"))
        if True:
            # ---- constants ----
            iota_t = cp.tile([128, D], f32, tag="iota")
            nc.sync.dma_start(iota_t[:], iota_d[:])
            id_t = cp.tile([128, D], f32, tag="ident")
            nc.sync.dma_start(id_t[:], ident_d[:])
            dvl_t = cp.tile([128, NW], f32, tag="dvl")
            nc.sync.dma_start(dvl_t[:], dinv_local_d[:])
            bat_t = cp.tile([128, NW], f32, tag="bat")
            nc.sync.dma_start(bat_t[:], batch_local_d[:])
            ci_t = cp.tile([G, 1], f32, tag="ci")
            nc.sync.dma_start(ci_t[:], cnt_inv_d[:])
            W_t = []
            for li in range(3):
                fo = DOUT if li == 2 else D
                wt = cp.tile([D, fo], f32, tag=f"W{li}", name=f"Wt{li}")
                nc.sync.dma_start(wt[:], W_d[li][:])
                W_t.append(wt)
            b3_t = cp.tile([DOUT, 1], f32, tag="b3")
            nc.sync.dma_start(b3_t[:], b3_d[:])
            gam_t, bet_t = [], []
            for li in range(2):
                g = cp.tile([D, 1], f32, tag=f"g{li}", name=f"gam{li}")
                nc.sync.dma_start(g[:], gam_d[li][:])
                gam_t.append(g)
                b = cp.tile([D, 1], f32, tag=f"be{li}", name=f"bet{li}")
                nc.sync.dma_start(b[:], bet_d[li][:])
                bet_t.append(b)

            # ---- DRAM internals ----
            tables = [dramp.tile([N, D], DT, tag=f"table{i}",
                                 name=f"table{i}", addr_space="Shared")
                      for i in range(3)]
            agin = dramp.tile([NLOC, D], DT, tag="agin")
            ar_b = [(dramp.tile([128, 2], f32, tag=f"ari{i}",
                                name=f"ari{i}"),
                     dramp.tile([128, 2], f32, tag=f"aro{i}",
                                name=f"aro{i}", addr_space="Shared"))
                    for i in range(2)]
            arp_i = dramp.tile([G, DOUT], f32, tag="arpi")
            arp_o = dramp.tile([G, DOUT], f32, tag="arpo",
                               addr_space="Shared")

            rg = [list(range(NCORES))]

            def prep_table(li, src_feat_major=None):
                """Write dinv-scaled node-major rows into agin, AllGather."""
                for w in range(NW):
                    cnt = min(128, NLOC - w * 128)
                    if src_feat_major is None:
                        xt = xpp.tile([128, D], f32, tag="xt")
                        nc.sync.dma_start(xt[:cnt, :],
                                          x_local[w * 128:w * 128 + cnt, :])
                        src_nm = xt
                    else:
                        tps = psW.tile([128, D], f32, tag="win",
                                       space="PSUM")
                        nc.tensor.transpose(
                            tps[:], src_feat_major[:D, w * 128:(w + 1) * 128],
                            id_t[:])
                        src_nm = tps
                    hq = xpp.tile([128, D], DT, tag="hq")
                    nc.scalar.activation(
                        hq[:cnt, :], src_nm[:cnt, :],
                        mybir.ActivationFunctionType.Copy,
                        bias=0.0, scale=dvl_t[:cnt, w:w + 1])
                    nc.sync.dma_start(agin[w * 128:w * 128 + cnt, :],
                                      hq[:cnt, :])
                if not nocc:
                    nc.gpsimd.collective_compute(
                        "AllGather", mybir.AluOpType.bypass,
                        replica_groups=rg, ins=[agin.opt()],
                        outs=[tables[li].opt()])

            prep_table(0, None)

            conv = None
            _lireq = {0: 1, 1: 3, 2: 4}
            for li in range(3):
                if ph < _lireq[li]:
                    break
                fo = DOUT if li == 2 else D
                conv = convp.tile([128, NWP], f32, tag="conv")
                for sbi, ws in enumerate(sbs):
                    ncols = len(ws) * 128
                    w0 = ws[0]
                    # stream idx + dstloc for this superblock
                    c16_0 = o16[(sbi, 0)][0]
                    c16_end = o16[(sbi, NCH - 1)][0] + \
                        o16[(sbi, NCH - 1)][1] // 16
                    idxt = idxp.tile([128, c16_end - c16_0], mybir.dt.int16,
                                     tag="idxt")
                    nc.sync.dma_start(idxt[:], idx16[:, c16_0:c16_end])
                    nb0 = int(blkoff[w0])
                    nb_sb = int(blkoff[ws[-1] + 1] - nb0)
                    dlt = dlp.tile([128, nb_sb], f32, tag="dlt")
                    nc.sync.dma_start(dlt[:], dstloc_d[:, nb0:nb0 + nb_sb])

                    msgs = []
                    for c in range(NCH):
                        c0, L = o16[(sbi, c)]
                        nblk_sc = L // 128
                        mt = msgp.tile([128, max(nblk_sc, 1), D], DT,
                                       tag="mt")
                        if L and "gather" not in kskip:
                            nc.gpsimd.dma_gather(
                                mt[:, :nblk_sc, :],
                                tables[li][c * CH:(c + 1) * CH, :],
                                idxt[:, c0 - c16_0:c0 - c16_0 + L // 16],
                                L, L, D, single_packet=False,
                                queue_num=c)
                        msgs.append(mt)

                    # dinv_dst broadcast [128, ncols]: partition-bcast DMA
                    dvsb = dvp.tile([128, ncols], f32, tag="dvsb")
                    nc.sync.dma_start(
                        dvsb[:],
                        dinv_row_d[0:1, w0 * 128:w0 * 128 + ncols]
                        .to_broadcast([128, ncols]))

                    aggT = aggp.tile([128, ncols], f32, tag="aggT")
                    for wi, w in enumerate(ws):
                        nbw = int(nblk_w[w])
                        if nbw == 0:
                            nc.vector.memset(aggT[:, wi * 128:(wi + 1) * 128],
                                             0.0)
                            continue
                        if "mm" in kskip:
                            nc.vector.memset(
                                aggT[:, wi * 128:(wi + 1) * 128], 0.0)
                            continue
                        rel = int(blkoff[w]) - nb0
                        Sw = Sp.tile([128, nbw, D], DT, tag="Sw")
                        nc.vector.tensor_tensor(
                            out=Sw[:],
                            in0=iota_t[:].rearrange("p (n f) -> p n f", n=1)
                                         .to_broadcast([128, nbw, D]),
                            in1=dlt[:, rel:rel + nbw]
                                .rearrange("p (n f) -> p n f", f=1)
                                .to_broadcast([128, nbw, D]),
                            op=mybir.AluOpType.is_equal)
                        ps = psW.tile([128, D], f32, tag="win", space="PSUM")
                        ops = []
                        for c in range(NCH):
                            bco = int(pad[[w2 for w2 in ws if w2 < w], c]
                                      .sum()) // 128 if ws else 0
                            for j in range(int(nblk_wc[w, c])):
                                ops.append((c, bco + j,
                                            int(cblk[w, c]) + j))
                        for k, (c, b, scol) in enumerate(ops):
                            nc.tensor.matmul(
                                ps[:], lhsT=msgs[c][:, b, :],
                                rhs=Sw[:, scol, :],
                                start=(k == 0), stop=(k == len(ops) - 1))
                        nc.vector.tensor_tensor(
                            out=aggT[:, wi * 128:(wi + 1) * 128],
                            in0=ps[:], in1=dvsb[:, wi * 128:(wi + 1) * 128],
                            op=mybir.AluOpType.mult)

                    gps = psG.tile([fo, ncols], f32, tag="gps", space="PSUM")
                    nc.tensor.matmul(gps[:], lhsT=W_t[li][:, :fo],
                                     rhs=aggT[:, :ncols],
                                     start=True, stop=True)
                    cc = w0 * 128
                    if li < 2:
                        nc.scalar.copy(conv[:fo, cc:cc + ncols], gps[:])
                    else:
                        nc.scalar.activation(
                            conv[:fo, cc:cc + ncols], gps[:],
                            mybir.ActivationFunctionType.Identity,
                            bias=b3_t[:, 0:1], scale=1.0)

                if li == 0 and ph < 2:
                    break
                if li < 2:
                    # ---- BatchNorm (global stats) + ReLU ----
                    stats = smlp.tile([128, 2], f32, tag="stats")
                    nc.vector.tensor_reduce(stats[:, 0:1], conv[:D, :NWP],
                                            mybir.AxisListType.X,
                                            mybir.AluOpType.add)
                    nchunk = (NWP + 511) // 512
                    sqcols = smlp.tile([128, nchunk], f32, tag="sqcols")
                    for k in range(nchunk):
                        a = k * 512
                        b = min(NWP, a + 512)
                        sq = sqp.tile([128, 512], f32, tag="sq")
                        nc.scalar.square(sq[:, :b - a], conv[:D, a:b])
                        nc.vector.tensor_reduce(
                            sqcols[:, k:k + 1], sq[:, :b - a],
                            mybir.AxisListType.X, mybir.AluOpType.add)
                    nc.vector.tensor_reduce(stats[:, 1:2], sqcols[:],
                                            mybir.AxisListType.X,
                                            mybir.AluOpType.add)
                    if ph == 20:
                        break
                    ari, aro = ar_b[li]
                    nc.sync.dma_start(ari[:], stats[:])
                    if not nocc:
                        nc.gpsimd.collective_compute(
                            "AllReduce", mybir.AluOpType.add,
                            replica_groups=rg, ins=[ari.opt()],
                            outs=[aro.opt()])
                    sg = smlp.tile([128, 2], f32, tag="sg")
                    nc.sync.dma_start(sg[:], aro[:])
                    if ph == 21:
                        break
                    mean = smlp.tile([128, 1], f32, tag="mean")
                    nc.vector.tensor_scalar(mean[:], sg[:, 0:1], 1.0 / N,
                                            None, mybir.AluOpType.mult)
                    ex2 = smlp.tile([128, 1], f32, tag="ex2")
                    nc.vector.tensor_scalar(ex2[:], sg[:, 1:2], 1.0 / N,
                                            None, mybir.AluOpType.mult)
                    var = smlp.tile([128, 1], f32, tag="var")
                    nc.vector.tensor_tensor(var[:], mean[:], mean[:],
                                            op=mybir.AluOpType.mult)
                    nc.vector.tensor_tensor(var[:], ex2[:], var[:],
                                            op=mybir.AluOpType.subtract)
                    nc.vector.tensor_scalar(var[:], var[:], EPS, None,
                                            mybir.AluOpType.add)
                    std = smlp.tile([128, 1], f32, tag="std")
                    nc.scalar.sqrt(std[:], var[:])
                    istd = smlp.tile([128, 1], f32, tag="istd")
                    nc.vector.reciprocal(istd[:], std[:])
                    sco = smlp.tile([128, 1], f32, tag="sco")
                    nc.vector.tensor_tensor(sco[:], gam_t[li][:], istd[:],
                                            op=mybir.AluOpType.mult)
                    sh = smlp.tile([128, 1], f32, tag="sh")
                    nc.vector.tensor_tensor(sh[:], mean[:], sco[:],
                                            op=mybir.AluOpType.mult)
                    nc.vector.tensor_tensor(sh[:], bet_t[li][:], sh[:],
                                            op=mybir.AluOpType.subtract)
                    nc.scalar.activation(conv[:D, :NWP], conv[:D, :NWP],
                                         mybir.ActivationFunctionType.Relu,
                                         bias=sh[:, 0:1], scale=sco[:, 0:1])
                    if ph == 22:
                        break
                    prep_table(li + 1, conv)
                elif ph >= 5:
                    # ---- global mean pool + sigmoid ----
                    pooled = psP.tile([G, DOUT], f32, tag="pooled",
                                      space="PSUM")
                    for w in range(NW):
                        Gw = gwp.tile([128, G], f32, tag="Gw")
                        nc.vector.tensor_tensor(
                            out=Gw[:], in0=iota_t[:, :G],
                            in1=bat_t[:, w:w + 1].to_broadcast([128, G]),
                            op=mybir.AluOpType.is_equal)
                        t3 = psW.tile([128, D], f32, tag="win",
                                      space="PSUM")
                        nc.tensor.transpose(
                            t3[:, :DOUT], conv[:DOUT, w * 128:(w + 1) * 128],
                            id_t[:DOUT, :DOUT])
                        c3 = gwp.tile([128, DOUT], f32, tag="c3")
                        nc.scalar.copy(c3[:], t3[:, :DOUT])
                        nc.tensor.matmul(pooled[:], lhsT=Gw[:], rhs=c3[:],
                                         start=(w == 0), stop=(w == NW - 1))
                    psb = smlp.tile([G, DOUT], f32, tag="psb")
                    nc.scalar.copy(psb[:], pooled[:])
                    nc.sync.dma_start(arp_i[:], psb[:])
                    if not nocc:
                        nc.gpsimd.collective_compute(
                            "AllReduce", mybir.AluOpType.add,
                            replica_groups=rg, ins=[arp_i.opt()],
                            outs=[arp_o.opt()])
                    pall = smlp.tile([G, DOUT], f32, tag="pall")
                    nc.sync.dma_start(pall[:], arp_o[:])
                    fin = smlp.tile([G, DOUT], f32, tag="fin")
                    nc.scalar.activation(
                        fin[:], pall[:],
                        mybir.ActivationFunctionType.Sigmoid,
                        bias=0.0, scale=ci_t[:, 0:1])
                    nc.sync.dma_start(out_d[:], fin[:])

    nc.compile()
    return nc


def prepare(x, edge_index, batch, W1, b1, W2, b2, W3, b3,
            gamma1, beta1, gamma2, beta2):
    """Build the Bass program + per-core input maps."""
    layout, per_core = _prep(np.asarray(x, np.float32), edge_index, batch)
    nc = _build(layout)

    iota = np.broadcast_to(np.arange(D, dtype=np.float32), (128, D)).copy()
    ident = np.eye(D, dtype=np.float32)
    shared = {
        "iota": iota, "ident": ident,
        "W1": np.asarray(W1, np.float32), "W2": np.asarray(W2, np.float32),
        "W3": np.asarray(W3, np.float32),
        "b3": np.asarray(b3, np.float32).reshape(DOUT, 1),
        "gamma1": np.asarray(gamma1, np.float32).reshape(D, 1),
        "gamma2": np.asarray(gamma2, np.float32).reshape(D, 1),
        "beta1": np.asarray(beta1, np.float32).reshape(D, 1),
        "beta2": np.asarray(beta2, np.float32).reshape(D, 1),
    }
    in_maps = []
    for r in range(NCORES):
        pc = per_core[r]
        in_maps.append({
            "x_local": pc["x_local"], "idx16": pc["idx16"],
            "dstloc": pc["dstloc"], "dinv_local": pc["dinv_local"],
            "dinv_row": pc["dinv_row"], "batch_local": pc["batch_local"],
            "cnt_inv": pc["cnt_inv"], **shared,
        })

    return nc, in_maps


def run_on_hw(nc, in_maps):
    from concourse.bass_utils import run_bass_kernel_spmd
    last = None
    for attempt in range(3):
        try:
            res = run_bass_kernel_spmd(nc, in_maps,
                                       core_ids=list(range(NCORES)))
            return np.asarray(res.results[0]["out"], np.float32)
        except Exception as e:  # transient device wedges happen
            last = e
    raise last


def kernel(x, edge_index, batch, W1, b1, W2, b2, W3, b3,
           gamma1, beta1, gamma2, beta2):
    nc, in_maps = prepare(x, edge_index, batch, W1, b1, W2, b2, W3, b3,
                          gamma1, beta1, gamma2, beta2)
    return run_on_hw(nc, in_maps)


if __name__ == "__main__":
    sys.path.insert(0, "/root/problem")
    import reference
    inputs = {k: np.asarray(v) for k, v in reference.setup_inputs().items()}
    out = kernel(**inputs)
    print("out", out.shape, out.dtype)


# revision 22
# speedup vs baseline: 1.3651x; 1.0153x over previous
"""GCN (3-layer GCNConv + BN/ReLU + global mean pool + sigmoid) on 8 trn2
NeuronCores via Bass/Tile.

Strategy (per sharding hint): 1D-partition the 100K nodes across 8 cores
(12500 each).  Edges (incl. self-loops) are bucketed by destination core /
128-node destination window / 25000-row source chunk on the host.  Each
layer: aggregate-first formulation  conv = diag(dinv) @ A_raw @ (diag(dinv)
@ h) @ W  computed as
  - dma_gather of scaled source rows h'[src] from a replicated (AllGather'd)
    node-major table in HBM; the 4 source chunks are issued on 4 SWDGE
    queues so descriptor generation runs on 4 Q7 cpu pairs in parallel,
  - segment-sum via TensorE matmuls against one-hot indicator matrices built
    on VectorE with a broadcast is_equal against an iota row,
  - per-dst dinv scaling (rank-1 PE broadcast of the dinv row),
  - dense GEMM with the replicated [d,d] weight,
  - BatchNorm with global stats via a tiny AllReduce, fused ReLU on ScalarE.
Graph mean-pool = indicator matmul against one-hot graph ids + AllReduce.
"""
import sys
sys.path.insert(0, "/opt/trn_rl_repo")

import numpy as np

N = 100000
E = 1600000
NCORES = 8
NLOC = N // NCORES          # 12500 nodes per core
D = 128
DOUT = 32
G = 64
NW = (NLOC + 127) // 128    # 98 windows (last has 84 nodes)
NWP = NW * 128              # 12544 padded local node slots
CH = 25000                  # source chunk rows (int16-indexable)
NCH = 4
SBW = 3                     # windows per superblock
EPS = 1e-5


def _ceil128(x):
    return (np.asarray(x) + 127) // 128 * 128


def _prep(x, edge_index, batch):
    """Host-side graph partitioning. Returns (layout, per_core_arrays)."""
    src0 = np.asarray(edge_index[0], dtype=np.int64)
    dst0 = np.asarray(edge_index[1], dtype=np.int64)
    loop = np.arange(N, dtype=np.int64)
    srcs = np.concatenate([src0, loop])
    dsts = np.concatenate([dst0, loop])

    deg = np.bincount(dsts, minlength=N).astype(np.float64)
    dinv = (1.0 / np.sqrt(np.maximum(deg, 1.0))).astype(np.float32)
    dinv[deg == 0] = 0.0

    core = dsts // NLOC
    nloc = dsts % NLOC
    win = nloc >> 7
    dl = (nloc & 127).astype(np.float32)
    ch = srcs // CH
    il = (srcs % CH).astype(np.int16)

    key = ((core * NW + win) * NCH + ch).astype(np.int64)
    order = np.argsort(key, kind="stable")
    il_s = il[order]
    dl_s = dl[order]
    cnts = np.bincount(key, minlength=NCORES * NW * NCH).reshape(
        NCORES, NW, NCH)
    starts = np.zeros(NCORES * NW * NCH + 1, np.int64)
    np.cumsum(cnts.ravel(), out=starts[1:])

    pad = _ceil128(cnts.max(axis=0)).astype(np.int64)   # [NW, NCH]
    nblk_wc = pad // 128                                # [NW, NCH]
    nblk_w = nblk_wc.sum(axis=1)                        # [NW]
    cblk = np.zeros((NW, NCH), np.int64)                # block off within win
    cblk[:, 1:] = np.cumsum(nblk_wc[:, :-1], axis=1)
    blkoff = np.zeros(NW + 1, np.int64)                 # global dstloc col off
    np.cumsum(nblk_w, out=blkoff[1:])
    nblk_tot = int(blkoff[-1])

    sbs = [list(range(i, min(i + SBW, NW))) for i in range(0, NW, SBW)]
    # idx16 column layout: per sb, per chunk call
    o16 = {}
    col16 = 0
    for sbi, ws in enumerate(sbs):
        for c in range(NCH):
            L = int(pad[ws, c].sum())
            o16[(sbi, c)] = (col16, L)
            col16 += L // 16
    cols16_tot = col16

    layout = dict(pad=pad, nblk_wc=nblk_wc, nblk_w=nblk_w, cblk=cblk,
                  blkoff=blkoff, nblk_tot=nblk_tot, sbs=sbs, o16=o16,
                  cols16_tot=cols16_tot)

    per_core = []
    batch = np.asarray(batch, dtype=np.int64)
    cnt_g = np.bincount(batch, minlength=G).astype(np.float32)
    cnt_inv = (1.0 / np.maximum(cnt_g, 1.0)).reshape(G, 1).astype(np.float32)

    for r in range(NCORES):
        idx16 = np.zeros((16, cols16_tot), np.int16)
        dstloc = np.full((128, nblk_tot), -1.0, np.float32)
        for sbi, ws in enumerate(sbs):
            for c in range(NCH):
                c0, L = o16[(sbi, c)]
                if L == 0:
                    continue
                flat = np.zeros(L, np.int16)
                q0 = 0
                for w in ws:
                    gk = (r * NW + w) * NCH + c
                    s = int(starts[gk])
                    n = int(cnts[r, w, c])
                    if n:
                        flat[q0:q0 + n] = il_s[s:s + n]
                        t = np.arange(n)
                        dstloc[t & 127,
                               blkoff[w] + cblk[w, c] + (t >> 7)] = \
                            dl_s[s:s + n]
                    q0 += int(pad[w, c])
                idx16[:, c0:c0 + L // 16] = flat.reshape(L // 16, 16).T
        idx16 = np.tile(idx16, (8, 1))  # [128, cols16_tot]

        nds = np.arange(NWP)
        gl = r * NLOC + nds
        valid = nds < NLOC
        dv = np.where(valid, dinv[np.minimum(gl, N - 1)], 0.0).astype(
            np.float32)
        dinv_local = dv.reshape(NW, 128).T.copy()          # [128, NW]
        dinv_row = dv.reshape(1, NWP).copy()               # [1, NWP]
        bl = np.where(valid, batch[np.minimum(gl, N - 1)], -1.0).astype(
            np.float32)
        batch_local = bl.reshape(NW, 128).T.copy()         # [128, NW]
        x_local = np.asarray(x[r * NLOC:(r + 1) * NLOC], np.float32)

        per_core.append(dict(idx16=idx16, dstloc=dstloc,
                             dinv_local=dinv_local, dinv_row=dinv_row,
                             batch_local=batch_local, x_local=x_local,
                             cnt_inv=cnt_inv))
    return layout, per_core


def _build(layout):
    import os
    import concourse.tile as tile
    from concourse import bacc, mybir

    ph = int(os.environ.get("KPHASE", "99"))
    nocc = bool(int(os.environ.get("KNOCC", "0")))
    kskip = os.environ.get("KSKIP", "")

    f32 = mybir.dt.float32
    DT = f32
    pad = layout["pad"]
    nblk_wc = layout["nblk_wc"]
    nblk_w = layout["nblk_w"]
    cblk = layout["cblk"]
    blkoff = layout["blkoff"]
    nblk_tot = layout["nblk_tot"]
    sbs = layout["sbs"]
    o16 = layout["o16"]
    cols16_tot = layout["cols16_tot"]

    nc = bacc.Bacc("TRN2", target_bir_lowering=False, debug=False,
                   num_devices=NCORES, num_swdge_queues=4)

    def din(name, shape, dt=f32):
        return nc.dram_tensor(name, shape, dt, kind="ExternalInput")

    x_local = din("x_local", [NLOC, D])
    idx16 = din("idx16", [128, cols16_tot], mybir.dt.int16)
    dstloc_d = din("dstloc", [128, nblk_tot])
    dinv_local_d = din("dinv_local", [128, NW])
    dinv_row_d = din("dinv_row", [1, NWP])
    batch_local_d = din("batch_local", [128, NW])
    cnt_inv_d = din("cnt_inv", [G, 1])
    iota_d = din("iota", [128, D])
    ident_d = din("ident", [128, D])
    W_d = [din("W1", [D, D]), din("W2", [D, D]), din("W3", [D, DOUT])]
    b3_d = din("b3", [DOUT, 1])
    gam_d = [din("gamma1", [D, 1]), din("gamma2", [D, 1])]
    bet_d = [din("beta1", [D, 1]), din("beta2", [D, 1])]
    out_d = nc.dram_tensor("out", [G, DOUT], f32, kind="ExternalOutput")

    from contextlib import ExitStack
    with tile.TileContext(nc) as tc, ExitStack() as _ctx:
        ec = _ctx.enter_context
        cp = ec(tc.tile_pool(name="const", bufs=1))
        convp = ec(tc.tile_pool(name="conv", bufs=1))
        xpp = ec(tc.tile_pool(name="xprep", bufs=4))
        idxp = ec(tc.tile_pool(name="idxs", bufs=3))
        dlp = ec(tc.tile_pool(name="dls", bufs=3))
        msgp = ec(tc.tile_pool(name="msg", bufs=10))
        Sp = ec(tc.tile_pool(name="Sp", bufs=2))
        aggp = ec(tc.tile_pool(name="agg", bufs=2))
        dvp = ec(tc.tile_pool(name="dv", bufs=2))
        smlp = ec(tc.tile_pool(name="sml", bufs=16))
        sqp = ec(tc.tile_pool(name="sq", bufs=2))
        gwp = ec(tc.tile_pool(name="gw", bufs=4))
        dramp = ec(tc.tile_pool(name="dram", bufs=1, space="DRAM"))
        # PSUM bank budget (8 banks): win/tp/t3 share 4 rotating slots,
        # dv 1, gemm 2, pooled 1.
        psW = ec(tc.tile_pool(name="psW", bufs=3, space="PSUM"))
        psG = ec(tc.tile_pool(name="psG", bufs=2, space="PSUM"))
        psP = ec(tc.tile_pool(name="psP", bufs=1, space="PSUM"))
        if True:
            # ---- constants ----
            iota_t = cp.tile([128, D], f32, tag="iota")
            nc.sync.dma_start(iota_t[:], iota_d[:])
            id_t = cp.tile([128, D], f32, tag="ident")
            nc.sync.dma_start(id_t[:], ident_d[:])
            dvl_t = cp.tile([128, NW], f32, tag="dvl")
            nc.sync.dma_start(dvl_t[:], dinv_local_d[:])
            bat_t = cp.tile([128, NW], f32, tag="bat")
            nc.sync.dma_start(bat_t[:], batch_local_d[:])
            ci_t = cp.tile([G, 1], f32, tag="ci")
            nc.sync.dma_start(ci_t[:], cnt_inv_d[:])
            W_t = []
            for li in range(3):
                fo = DOUT if li == 2 else D
                wt = cp.tile([D, fo], f32, tag=f"W{li}", name=f"Wt{li}")
                nc.sync.dma_start(wt[:], W_d[li][:])
                W_t.append(wt)
            b3_t = cp.tile([DOUT, 1], f32, tag="b3")
            nc.sync.dma_start(b3_t[:], b3_d[:])
            gam_t, bet_t = [], []
            for li in range(2):
                g = cp.tile([D, 1], f32, tag=f"g{li}", name=f"gam{li}")
                nc.sync.dma_start(g[:], gam_d[li][:])
                gam_t.append(g)
                b = cp.tile([D, 1], f32, tag=f"be{li}", name=f"bet{li}")
                nc.sync.dma_start(b[:], bet_d[li][:])
                bet_t.append(b)

            # ---- DRAM internals ----
            tables = [dramp.tile([N, D], DT, tag=f"table{i}",
                                 name=f"table{i}", addr_space="Shared")
                      for i in range(3)]
            agin = dramp.tile([NLOC, D], DT, tag="agin")
            ar_b = [(dramp.tile([128, 2], f32, tag=f"ari{i}",
                                name=f"ari{i}"),
                     dramp.tile([128, 2], f32, tag=f"aro{i}",
                                name=f"aro{i}", addr_space="Shared"))
                    for i in range(2)]
            arp_i = dramp.tile([G, DOUT], f32, tag="arpi")
            arp_o = dramp.tile([G, DOUT], f32, tag="arpo",
                               addr_space="Shared")

            rg = [list(range(NCORES))]

            def prep_table(li, src_feat_major=None):
                """Write dinv-scaled node-major rows into agin, AllGather."""
                for w in range(NW):
                    cnt = min(128, NLOC - w * 128)
                    if src_feat_major is None:
                        xt = xpp.tile([128, D], f32, tag="xt")
                        nc.sync.dma_start(xt[:cnt, :],
                                          x_local[w * 128:w * 128 + cnt, :])
                        src_nm = xt
                    else:
                        tps = psW.tile([128, D], f32, tag="win",
                                       space="PSUM")
                        nc.tensor.transpose(
                            tps[:], src_feat_major[:D, w * 128:(w + 1) * 128],
                            id_t[:])
                        src_nm = tps
                    hq = xpp.tile([128, D], DT, tag="hq")
                    nc.scalar.activation(
                        hq[:cnt, :], src_nm[:cnt, :],
                        mybir.ActivationFunctionType.Copy,
                        bias=0.0, scale=dvl_t[:cnt, w:w + 1])
                    nc.sync.dma_start(agin[w * 128:w * 128 + cnt, :],
                                      hq[:cnt, :])
                if not nocc:
                    nc.gpsimd.collective_compute(
                        "AllGather", mybir.AluOpType.bypass,
                        replica_groups=rg, ins=[agin.opt()],
                        outs=[tables[li].opt()])

            prep_table(0, None)

            conv = None
            _lireq = {0: 1, 1: 3, 2: 4}
            for li in range(3):
                if ph < _lireq[li]:
                    break
                fo = DOUT if li == 2 else D
                conv = convp.tile([128, NWP], f32, tag="conv")
                if li < 2:
                    scols = smlp.tile([128, len(sbs)], f32, tag=f"scol{li}",
                                      name=f"scol{li}", bufs=1)
                    qcols = smlp.tile([128, len(sbs)], f32, tag=f"qcol{li}",
                                      name=f"qcol{li}", bufs=1)
                for sbi, ws in enumerate(sbs):
                    ncols = len(ws) * 128
                    w0 = ws[0]
                    # stream idx + dstloc for this superblock
                    c16_0 = o16[(sbi, 0)][0]
                    c16_end = o16[(sbi, NCH - 1)][0] + \
                        o16[(sbi, NCH - 1)][1] // 16
                    idxt = idxp.tile([128, c16_end - c16_0], mybir.dt.int16,
                                     tag="idxt")
                    nc.sync.dma_start(idxt[:], idx16[:, c16_0:c16_end])
                    nb0 = int(blkoff[w0])
                    nb_sb = int(blkoff[ws[-1] + 1] - nb0)
                    dlt = dlp.tile([128, nb_sb], f32, tag="dlt")
                    nc.sync.dma_start(dlt[:], dstloc_d[:, nb0:nb0 + nb_sb])

                    msgs = []
                    for c in range(NCH):
                        c0, L = o16[(sbi, c)]
                        nblk_sc = L // 128
                        mt = msgp.tile([128, max(nblk_sc, 1), D], DT,
                                       tag="mt")
                        if L and "gather" not in kskip:
                            nc.gpsimd.dma_gather(
                                mt[:, :nblk_sc, :],
                                tables[li][c * CH:(c + 1) * CH, :],
                                idxt[:, c0 - c16_0:c0 - c16_0 + L // 16],
                                L, L, D, single_packet=False,
                                queue_num=c)
                        msgs.append(mt)

                    # dinv_dst broadcast [128, ncols]: partition-bcast DMA
                    dvsb = dvp.tile([128, ncols], f32, tag="dvsb")
                    nc.sync.dma_start(
                        dvsb[:],
                        dinv_row_d[0:1, w0 * 128:w0 * 128 + ncols]
                        .to_broadcast([128, ncols]))

                    aggT = aggp.tile([128, ncols], f32, tag="aggT")
                    for wi, w in enumerate(ws):
                        nbw = int(nblk_w[w])
                        if nbw == 0:
                            nc.vector.memset(aggT[:, wi * 128:(wi + 1) * 128],
                                             0.0)
                            continue
                        if "mm" in kskip:
                            nc.vector.memset(
                                aggT[:, wi * 128:(wi + 1) * 128], 0.0)
                            continue
                        rel = int(blkoff[w]) - nb0
                        Sw = Sp.tile([128, nbw, D], DT, tag="Sw")
                        nc.vector.tensor_tensor(
                            out=Sw[:],
                            in0=iota_t[:].rearrange("p (n f) -> p n f", n=1)
                                         .to_broadcast([128, nbw, D]),
                            in1=dlt[:, rel:rel + nbw]
                                .rearrange("p (n f) -> p n f", f=1)
                                .to_broadcast([128, nbw, D]),
                            op=mybir.AluOpType.is_equal)
                        ps = psW.tile([128, D], f32, tag="win", space="PSUM")
                        ops = []
                        for c in range(NCH):
                            bco = int(pad[[w2 for w2 in ws if w2 < w], c]
                                      .sum()) // 128 if ws else 0
                            for j in range(int(nblk_wc[w, c])):
                                ops.append((c, bco + j,
                                            int(cblk[w, c]) + j))
                        for k, (c, b, scol) in enumerate(ops):
                            nc.tensor.matmul(
                                ps[:], lhsT=msgs[c][:, b, :],
                                rhs=Sw[:, scol, :],
                                start=(k == 0), stop=(k == len(ops) - 1))
                        nc.vector.tensor_tensor(
                            out=aggT[:, wi * 128:(wi + 1) * 128],
                            in0=ps[:], in1=dvsb[:, wi * 128:(wi + 1) * 128],
                            op=mybir.AluOpType.mult)

                    gps = psG.tile([fo, ncols], f32, tag="gps", space="PSUM")
                    nc.tensor.matmul(gps[:], lhsT=W_t[li][:, :fo],
                                     rhs=aggT[:, :ncols],
                                     start=True, stop=True)
                    cc = w0 * 128
                    if li < 2:
                        nc.scalar.copy(conv[:fo, cc:cc + ncols], gps[:])
                        sqs = sqp.tile([128, SBW * 128], f32, tag="sqs")
                        nc.scalar.square(sqs[:, :ncols],
                                         conv[:D, cc:cc + ncols])
                        nc.vector.tensor_reduce(
                            qcols[:, sbi:sbi + 1], sqs[:, :ncols],
                            mybir.AxisListType.X, mybir.AluOpType.add)
                        nc.vector.tensor_reduce(
                            scols[:, sbi:sbi + 1], conv[:D, cc:cc + ncols],
                            mybir.AxisListType.X, mybir.AluOpType.add)
                    else:
                        nc.scalar.activation(
                            conv[:fo, cc:cc + ncols], gps[:],
                            mybir.ActivationFunctionType.Identity,
                            bias=b3_t[:, 0:1], scale=1.0)

                if li == 0 and ph < 2:
                    break
                if li < 2:
                    # ---- BatchNorm global stats (accumulated per-sb) ----
                    stats = smlp.tile([128, 2], f32, tag="stats")
                    nc.vector.tensor_reduce(stats[:, 0:1], scols[:],
                                            mybir.AxisListType.X,
                                            mybir.AluOpType.add)
                    nc.vector.tensor_reduce(stats[:, 1:2], qcols[:],
                                            mybir.AxisListType.X,
                                            mybir.AluOpType.add)
                    if ph == 20:
                        break
                    ari, aro = ar_b[li]
                    nc.sync.dma_start(ari[:], stats[:])
                    if not nocc:
                        nc.gpsimd.collective_compute(
                            "AllReduce", mybir.AluOpType.add,
                            replica_groups=rg, ins=[ari.opt()],
                            outs=[aro.opt()])
                    sg = smlp.tile([128, 2], f32, tag="sg")
                    nc.sync.dma_start(sg[:], aro[:])
                    if ph == 21:
                        break
                    mean = smlp.tile([128, 1], f32, tag="mean")
                    nc.vector.tensor_scalar(mean[:], sg[:, 0:1], 1.0 / N,
                                            None, mybir.AluOpType.mult)
                    ex2 = smlp.tile([128, 1], f32, tag="ex2")
                    nc.vector.tensor_scalar(ex2[:], sg[:, 1:2], 1.0 / N,
                                            None, mybir.AluOpType.mult)
                    var = smlp.tile([128, 1], f32, tag="var")
                    nc.vector.tensor_tensor(var[:], mean[:], mean[:],
                                            op=mybir.AluOpType.mult)
                    nc.vector.tensor_tensor(var[:], ex2[:], var[:],
                                            op=mybir.AluOpType.subtract)
                    nc.vector.tensor_scalar(var[:], var[:], EPS, None,
                                            mybir.AluOpType.add)
                    std = smlp.tile([128, 1], f32, tag="std")
                    nc.scalar.sqrt(std[:], var[:])
                    istd = smlp.tile([128, 1], f32, tag="istd")
                    nc.vector.reciprocal(istd[:], std[:])
                    sco = smlp.tile([128, 1], f32, tag="sco")
                    nc.vector.tensor_tensor(sco[:], gam_t[li][:], istd[:],
                                            op=mybir.AluOpType.mult)
                    sh = smlp.tile([128, 1], f32, tag="sh")
                    nc.vector.tensor_tensor(sh[:], mean[:], sco[:],
                                            op=mybir.AluOpType.mult)
                    nc.vector.tensor_tensor(sh[:], bet_t[li][:], sh[:],
                                            op=mybir.AluOpType.subtract)
                    nc.scalar.activation(conv[:D, :NWP], conv[:D, :NWP],
                                         mybir.ActivationFunctionType.Relu,
                                         bias=sh[:, 0:1], scale=sco[:, 0:1])
                    if ph == 22:
                        break
                    prep_table(li + 1, conv)
                elif ph >= 5:
                    # ---- global mean pool + sigmoid ----
                    pooled = psP.tile([G, DOUT], f32, tag="pooled",
                                      space="PSUM")
                    for w in range(NW):
                        Gw = gwp.tile([128, G], f32, tag="Gw")
                        nc.vector.tensor_tensor(
                            out=Gw[:], in0=iota_t[:, :G],
                            in1=bat_t[:, w:w + 1].to_broadcast([128, G]),
                            op=mybir.AluOpType.is_equal)
                        t3 = psW.tile([128, D], f32, tag="win",
                                      space="PSUM")
                        nc.tensor.transpose(
                            t3[:, :DOUT], conv[:DOUT, w * 128:(w + 1) * 128],
                            id_t[:DOUT, :DOUT])
                        c3 = gwp.tile([128, DOUT], f32, tag="c3")
                        nc.scalar.copy(c3[:], t3[:, :DOUT])
                        nc.tensor.matmul(pooled[:], lhsT=Gw[:], rhs=c3[:],
                                         start=(w == 0), stop=(w == NW - 1))
                    psb = smlp.tile([G, DOUT], f32, tag="psb")
                    nc.scalar.copy(psb[:], pooled[:])
                    nc.sync.dma_start(arp_i[:], psb[:])
                    if not nocc:
                        nc.gpsimd.collective_compute(
                            "AllReduce", mybir.AluOpType.add,
                            replica_groups=rg, ins=[arp_i.opt()],
                            outs=[arp_o.opt()])
                    pall = smlp.tile([G, DOUT], f32, tag="pall")
                    nc.sync.dma_start(pall[:], arp_o[:])
                    fin = smlp.tile([G, DOUT], f32, tag="fin")
                    nc.scalar.activation(
                        fin[:], pall[:],
                        mybir.ActivationFunctionType.Sigmoid,
                        bias=0.0, scale=ci_t[:, 0:1])
                    nc.sync.dma_start(out_d[:], fin[:])

    nc.compile()
    return nc


def prepare(x, edge_index, batch, W1, b1, W2, b2, W3, b3,
            gamma1, beta1, gamma2, beta2):
    """Build the Bass program + per-core input maps."""
    layout, per_core = _prep(np.asarray(x, np.float32), edge_index, batch)
    nc = _build(layout)

    iota = np.broadcast_to(np.arange(D, dtype=np.float32), (128, D)).copy()
    ident = np.eye(D, dtype=np.float32)
    shared = {
        "iota": iota, "ident": ident,
        "W1": np.asarray(W1, np.float32), "W2": np.asarray(W2, np.float32),
        "W3": np.asarray(W3, np.float32),
        "b3": np.asarray(b3, np.float32).reshape(DOUT, 1),
        "gamma1": np.asarray(gamma1, np.float32).reshape(D, 1),
        "gamma2": np.asarray(gamma2, np.float32).reshape(D, 1),
        "beta1": np.asarray(beta1, np.float32).reshape(D, 1),
        "beta2": np.asarray(beta2, np.float32).reshape(D, 1),
    }
    in_maps = []
    for r in range(NCORES):
        pc = per_core[r]
        in_maps.append({
            "x_local": pc["x_local"], "idx16": pc["idx16"],
            "dstloc": pc["dstloc"], "dinv_local": pc["dinv_local"],
            "dinv_row": pc["dinv_row"], "batch_local": pc["batch_local"],
            "cnt_inv": pc["cnt_inv"], **shared,
        })

    return nc, in_maps


def run_on_hw(nc, in_maps):
    from concourse.bass_utils import run_bass_kernel_spmd
    last = None
    for attempt in range(3):
        try:
            res = run_bass_kernel_spmd(nc, in_maps,
                                       core_ids=list(range(NCORES)))
            return np.asarray(res.results[0]["out"], np.float32)
        except Exception as e:  # transient device wedges happen
            last = e
    raise last


def kernel(x, edge_index, batch, W1, b1, W2, b2, W3, b3,
           gamma1, beta1, gamma2, beta2):
    nc, in_maps = prepare(x, edge_index, batch, W1, b1, W2, b2, W3, b3,
                          gamma1, beta1, gamma2, beta2)
    return run_on_hw(nc, in_maps)


if __name__ == "__main__":
    sys.path.insert(0, "/root/problem")
    import reference
    inputs = {k: np.asarray(v) for k, v in reference.setup_inputs().items()}
    out = kernel(**inputs)
    print("out", out.shape, out.dtype)


# revision 23
# speedup vs baseline: 1.4124x; 1.0347x over previous
"""GCN (3-layer GCNConv + BN/ReLU + global mean pool + sigmoid) on 8 trn2
NeuronCores via Bass/Tile.

Strategy (per sharding hint): 1D-partition the 100K nodes across 8 cores
(12500 each).  Edges (incl. self-loops) are bucketed by destination core /
128-node destination window / 25000-row source chunk on the host.  Each
layer: aggregate-first formulation  conv = diag(dinv) @ A_raw @ (diag(dinv)
@ h) @ W  computed as
  - dma_gather of scaled source rows h'[src] from a replicated (AllGather'd)
    node-major table in HBM; the 4 source chunks are issued on 4 SWDGE
    queues so descriptor generation runs on 4 Q7 cpu pairs in parallel,
  - segment-sum via TensorE matmuls against one-hot indicator matrices built
    on VectorE with a broadcast is_equal against an iota row,
  - per-dst dinv scaling (rank-1 PE broadcast of the dinv row),
  - dense GEMM with the replicated [d,d] weight,
  - BatchNorm with global stats via a tiny AllReduce, fused ReLU on ScalarE.
Graph mean-pool = indicator matmul against one-hot graph ids + AllReduce.
"""
import sys
sys.path.insert(0, "/opt/trn_rl_repo")

import numpy as np

N = 100000
E = 1600000
NCORES = 8
NLOC = N // NCORES          # 12500 nodes per core
D = 128
DOUT = 32
G = 64
NW = (NLOC + 127) // 128    # 98 windows (last has 84 nodes)
NWP = NW * 128              # 12544 padded local node slots
CH = 25000                  # source chunk rows (int16-indexable)
NCH = 4
SBW = 3                     # windows per superblock
EPS = 1e-5


def _ceil128(x):
    return (np.asarray(x) + 127) // 128 * 128


def _prep(x, edge_index, batch):
    """Host-side graph partitioning. Returns (layout, per_core_arrays)."""
    src0 = np.asarray(edge_index[0], dtype=np.int64)
    dst0 = np.asarray(edge_index[1], dtype=np.int64)
    loop = np.arange(N, dtype=np.int64)
    srcs = np.concatenate([src0, loop])
    dsts = np.concatenate([dst0, loop])

    deg = np.bincount(dsts, minlength=N).astype(np.float64)
    dinv = (1.0 / np.sqrt(np.maximum(deg, 1.0))).astype(np.float32)
    dinv[deg == 0] = 0.0

    core = dsts // NLOC
    nloc = dsts % NLOC
    win = nloc >> 7
    dl = (nloc & 127).astype(np.float32)
    HH = NLOC // 2                       # 6250 rows per core per half
    r_src = srcs // NLOC
    j_src = srcs % NLOC
    half = (j_src >= HH).astype(np.int64)
    rowh = r_src * HH + (j_src - half * HH)   # row within the half-table
    ch = half * 2 + rowh // CH                # chunks 0,1 -> A; 2,3 -> B
    il = (rowh % CH).astype(np.int16)

    key = ((core * NW + win) * NCH + ch).astype(np.int64)
    order = np.argsort(key, kind="stable")
    il_s = il[order]
    dl_s = dl[order]
    cnts = np.bincount(key, minlength=NCORES * NW * NCH).reshape(
        NCORES, NW, NCH)
    starts = np.zeros(NCORES * NW * NCH + 1, np.int64)
    np.cumsum(cnts.ravel(), out=starts[1:])

    pad = _ceil128(cnts.max(axis=0)).astype(np.int64)   # [NW, NCH]
    nblk_wc = pad // 128                                # [NW, NCH]
    nblk_w = nblk_wc.sum(axis=1)                        # [NW]
    cblk = np.zeros((NW, NCH), np.int64)                # block off within win
    cblk[:, 1:] = np.cumsum(nblk_wc[:, :-1], axis=1)
    blkoff = np.zeros(NW + 1, np.int64)                 # global dstloc col off
    np.cumsum(nblk_w, out=blkoff[1:])
    nblk_tot = int(blkoff[-1])

    sbs = [list(range(i, min(i + SBW, NW))) for i in range(0, NW, SBW)]
    # idx16 column layout: per sb, per chunk call
    o16 = {}
    col16 = 0
    for sbi, ws in enumerate(sbs):
        for c in range(NCH):
            L = int(pad[ws, c].sum())
            o16[(sbi, c)] = (col16, L)
            col16 += L // 16
    cols16_tot = col16

    layout = dict(pad=pad, nblk_wc=nblk_wc, nblk_w=nblk_w, cblk=cblk,
                  blkoff=blkoff, nblk_tot=nblk_tot, sbs=sbs, o16=o16,
                  cols16_tot=cols16_tot)

    per_core = []
    batch = np.asarray(batch, dtype=np.int64)
    cnt_g = np.bincount(batch, minlength=G).astype(np.float32)
    cnt_inv = (1.0 / np.maximum(cnt_g, 1.0)).reshape(G, 1).astype(np.float32)

    for r in range(NCORES):
        idx16 = np.zeros((16, cols16_tot), np.int16)
        dstloc = np.full((128, nblk_tot), -1.0, np.float32)
        for sbi, ws in enumerate(sbs):
            for c in range(NCH):
                c0, L = o16[(sbi, c)]
                if L == 0:
                    continue
                flat = np.zeros(L, np.int16)
                q0 = 0
                for w in ws:
                    gk = (r * NW + w) * NCH + c
                    s = int(starts[gk])
                    n = int(cnts[r, w, c])
                    if n:
                        flat[q0:q0 + n] = il_s[s:s + n]
                        t = np.arange(n)
                        dstloc[t & 127,
                               blkoff[w] + cblk[w, c] + (t >> 7)] = \
                            dl_s[s:s + n]
                    q0 += int(pad[w, c])
                idx16[:, c0:c0 + L // 16] = flat.reshape(L // 16, 16).T
        idx16 = np.tile(idx16, (8, 1))  # [128, cols16_tot]

        nds = np.arange(NWP)
        gl = r * NLOC + nds
        valid = nds < NLOC
        dv = np.where(valid, dinv[np.minimum(gl, N - 1)], 0.0).astype(
            np.float32)
        dinv_local = dv.reshape(NW, 128).T.copy()          # [128, NW]
        dinv_row = dv.reshape(1, NWP).copy()               # [1, NWP]
        bl = np.where(valid, batch[np.minimum(gl, N - 1)], -1.0).astype(
            np.float32)
        batch_local = bl.reshape(NW, 128).T.copy()         # [128, NW]
        x_local = np.asarray(x[r * NLOC:(r + 1) * NLOC], np.float32)

        per_core.append(dict(idx16=idx16, dstloc=dstloc,
                             dinv_local=dinv_local, dinv_row=dinv_row,
                             batch_local=batch_local, x_local=x_local,
                             cnt_inv=cnt_inv))
    return layout, per_core


def _build(layout):
    import os
    import concourse.tile as tile
    from concourse import bacc, mybir

    ph = int(os.environ.get("KPHASE", "99"))
    nocc = bool(int(os.environ.get("KNOCC", "0")))
    kskip = os.environ.get("KSKIP", "")

    f32 = mybir.dt.float32
    DT = f32
    pad = layout["pad"]
    nblk_wc = layout["nblk_wc"]
    nblk_w = layout["nblk_w"]
    cblk = layout["cblk"]
    blkoff = layout["blkoff"]
    nblk_tot = layout["nblk_tot"]
    sbs = layout["sbs"]
    o16 = layout["o16"]
    cols16_tot = layout["cols16_tot"]

    nc = bacc.Bacc("TRN2", target_bir_lowering=False, debug=False,
                   num_devices=NCORES, num_swdge_queues=4)

    def din(name, shape, dt=f32):
        return nc.dram_tensor(name, shape, dt, kind="ExternalInput")

    x_local = din("x_local", [NLOC, D])
    idx16 = din("idx16", [128, cols16_tot], mybir.dt.int16)
    dstloc_d = din("dstloc", [128, nblk_tot])
    dinv_local_d = din("dinv_local", [128, NW])
    dinv_row_d = din("dinv_row", [1, NWP])
    batch_local_d = din("batch_local", [128, NW])
    cnt_inv_d = din("cnt_inv", [G, 1])
    iota_d = din("iota", [128, D])
    ident_d = din("ident", [128, D])
    W_d = [din("W1", [D, D]), din("W2", [D, D]), din("W3", [D, DOUT])]
    b3_d = din("b3", [DOUT, 1])
    gam_d = [din("gamma1", [D, 1]), din("gamma2", [D, 1])]
    bet_d = [din("beta1", [D, 1]), din("beta2", [D, 1])]
    out_d = nc.dram_tensor("out", [G, DOUT], f32, kind="ExternalOutput")

    from contextlib import ExitStack
    with tile.TileContext(nc) as tc, ExitStack() as _ctx:
        ec = _ctx.enter_context
        cp = ec(tc.tile_pool(name="const", bufs=1))
        convp = ec(tc.tile_pool(name="conv", bufs=1))
        xpp = ec(tc.tile_pool(name="xprep", bufs=4))
        idxp = ec(tc.tile_pool(name="idxs", bufs=3))
        dlp = ec(tc.tile_pool(name="dls", bufs=3))
        msgp = ec(tc.tile_pool(name="msg", bufs=10))
        Sp = ec(tc.tile_pool(name="Sp", bufs=2))
        aggp = ec(tc.tile_pool(name="agg", bufs=2))
        dvp = ec(tc.tile_pool(name="dv", bufs=2))
        smlp = ec(tc.tile_pool(name="sml", bufs=16))
        sqp = ec(tc.tile_pool(name="sq", bufs=2))
        gwp = ec(tc.tile_pool(name="gw", bufs=4))
        dramp = ec(tc.tile_pool(name="dram", bufs=1, space="DRAM"))
        # PSUM bank budget (8 banks): win/tp/t3 share 4 rotating slots,
        # dv 1, gemm 2, pooled 1.
        psW = ec(tc.tile_pool(name="psW", bufs=3, space="PSUM"))
        psG = ec(tc.tile_pool(name="psG", bufs=2, space="PSUM"))
        psP = ec(tc.tile_pool(name="psP", bufs=1, space="PSUM"))
        if True:
            # ---- constants ----
            iota_t = cp.tile([128, D], f32, tag="iota")
            nc.sync.dma_start(iota_t[:], iota_d[:])
            id_t = cp.tile([128, D], f32, tag="ident")
            nc.sync.dma_start(id_t[:], ident_d[:])
            dvl_t = cp.tile([128, NW], f32, tag="dvl")
            nc.sync.dma_start(dvl_t[:], dinv_local_d[:])
            bat_t = cp.tile([128, NW], f32, tag="bat")
            nc.sync.dma_start(bat_t[:], batch_local_d[:])
            ci_t = cp.tile([G, 1], f32, tag="ci")
            nc.sync.dma_start(ci_t[:], cnt_inv_d[:])
            W_t = []
            for li in range(3):
                fo = DOUT if li == 2 else D
                wt = cp.tile([D, fo], f32, tag=f"W{li}", name=f"Wt{li}")
                nc.sync.dma_start(wt[:], W_d[li][:])
                W_t.append(wt)
            b3_t = cp.tile([DOUT, 1], f32, tag="b3")
            nc.sync.dma_start(b3_t[:], b3_d[:])
            gam_t, bet_t = [], []
            for li in range(2):
                g = cp.tile([D, 1], f32, tag=f"g{li}", name=f"gam{li}")
                nc.sync.dma_start(g[:], gam_d[li][:])
                gam_t.append(g)
                b = cp.tile([D, 1], f32, tag=f"be{li}", name=f"bet{li}")
                nc.sync.dma_start(b[:], bet_d[li][:])
                bet_t.append(b)

            # ---- DRAM internals ----
            tables = [[dramp.tile([N // 2, D], DT, tag=f"table{i}{h}",
                                  name=f"table{i}{h}", addr_space="Shared")
                       for h in range(2)] for i in range(3)]
            HH = NLOC // 2
            agins = [dramp.tile([HH, D], DT, tag=f"agin{h}",
                                name=f"agin{h}") for h in range(2)]
            ar_b = [(dramp.tile([128, 2], f32, tag=f"ari{i}",
                                name=f"ari{i}"),
                     dramp.tile([128, 2], f32, tag=f"aro{i}",
                                name=f"aro{i}", addr_space="Shared"))
                    for i in range(2)]
            arp_i = dramp.tile([G, DOUT], f32, tag="arpi")
            arp_o = dramp.tile([G, DOUT], f32, tag="arpo",
                               addr_space="Shared")

            rg = [list(range(NCORES))]

            def prep_table(li, src_feat_major=None):
                """Write dinv-scaled node-major rows into the two agin
                halves; AllGather each half as soon as it completes so the
                second AG (and the first gathers) overlap the rest."""
                for w in range(NW):
                    cnt = min(128, NLOC - w * 128)
                    if src_feat_major is None:
                        xt = xpp.tile([128, D], f32, tag="xt")
                        nc.sync.dma_start(xt[:cnt, :],
                                          x_local[w * 128:w * 128 + cnt, :])
                        src_nm = xt
                    else:
                        tps = psW.tile([128, D], f32, tag="win",
                                       space="PSUM")
                        nc.tensor.transpose(
                            tps[:], src_feat_major[:D, w * 128:(w + 1) * 128],
                            id_t[:])
                        src_nm = tps
                    hq = xpp.tile([128, D], DT, tag="hq")
                    nc.scalar.activation(
                        hq[:cnt, :], src_nm[:cnt, :],
                        mybir.ActivationFunctionType.Copy,
                        bias=0.0, scale=dvl_t[:cnt, w:w + 1])
                    a = w * 128
                    b = a + cnt
                    if b <= HH:
                        nc.sync.dma_start(agins[0][a:b, :], hq[:cnt, :])
                    elif a >= HH:
                        nc.sync.dma_start(agins[1][a - HH:b - HH, :],
                                          hq[:cnt, :])
                    else:
                        s0 = HH - a
                        nc.sync.dma_start(agins[0][a:HH, :], hq[:s0, :])
                        nc.sync.dma_start(agins[1][0:b - HH, :],
                                          hq[s0:cnt, :])
                    if b >= HH and a < HH and not nocc:
                        nc.gpsimd.collective_compute(
                            "AllGather", mybir.AluOpType.bypass,
                            replica_groups=rg, ins=[agins[0].opt()],
                            outs=[tables[li][0].opt()])
                if not nocc:
                    nc.gpsimd.collective_compute(
                        "AllGather", mybir.AluOpType.bypass,
                        replica_groups=rg, ins=[agins[1].opt()],
                        outs=[tables[li][1].opt()])

            prep_table(0, None)

            conv = None
            _lireq = {0: 1, 1: 3, 2: 4}
            for li in range(3):
                if ph < _lireq[li]:
                    break
                fo = DOUT if li == 2 else D
                conv = convp.tile([128, NWP], f32, tag="conv")
                if li < 2:
                    scols = smlp.tile([128, len(sbs)], f32, tag=f"scol{li}",
                                      name=f"scol{li}", bufs=1)
                    qcols = smlp.tile([128, len(sbs)], f32, tag=f"qcol{li}",
                                      name=f"qcol{li}", bufs=1)
                for sbi, ws in enumerate(sbs):
                    ncols = len(ws) * 128
                    w0 = ws[0]
                    # stream idx + dstloc for this superblock
                    c16_0 = o16[(sbi, 0)][0]
                    c16_end = o16[(sbi, NCH - 1)][0] + \
                        o16[(sbi, NCH - 1)][1] // 16
                    idxt = idxp.tile([128, c16_end - c16_0], mybir.dt.int16,
                                     tag="idxt")
                    nc.sync.dma_start(idxt[:], idx16[:, c16_0:c16_end])
                    nb0 = int(blkoff[w0])
                    nb_sb = int(blkoff[ws[-1] + 1] - nb0)
                    dlt = dlp.tile([128, nb_sb], f32, tag="dlt")
                    nc.sync.dma_start(dlt[:], dstloc_d[:, nb0:nb0 + nb_sb])

                    msgs = []
                    for c in range(NCH):
                        c0, L = o16[(sbi, c)]
                        nblk_sc = L // 128
                        mt = msgp.tile([128, max(nblk_sc, 1), D], DT,
                                       tag="mt")
                        if L and "gather" not in kskip:
                            nc.gpsimd.dma_gather(
                                mt[:, :nblk_sc, :],
                                tables[li][c // 2][(c % 2) * CH:
                                                   (c % 2 + 1) * CH, :],
                                idxt[:, c0 - c16_0:c0 - c16_0 + L // 16],
                                L, L, D, single_packet=False,
                                queue_num=c)
                        msgs.append(mt)

                    # dinv_dst broadcast [128, ncols]: partition-bcast DMA
                    dvsb = dvp.tile([128, ncols], f32, tag="dvsb")
                    nc.sync.dma_start(
                        dvsb[:],
                        dinv_row_d[0:1, w0 * 128:w0 * 128 + ncols]
                        .to_broadcast([128, ncols]))

                    aggT = aggp.tile([128, ncols], f32, tag="aggT")
                    for wi, w in enumerate(ws):
                        nbw = int(nblk_w[w])
                        if nbw == 0:
                            nc.vector.memset(aggT[:, wi * 128:(wi + 1) * 128],
                                             0.0)
                            continue
                        if "mm" in kskip:
                            nc.vector.memset(
                                aggT[:, wi * 128:(wi + 1) * 128], 0.0)
                            continue
                        rel = int(blkoff[w]) - nb0
                        Sw = Sp.tile([128, nbw, D], DT, tag="Sw")
                        nc.vector.tensor_tensor(
                            out=Sw[:],
                            in0=iota_t[:].rearrange("p (n f) -> p n f", n=1)
                                         .to_broadcast([128, nbw, D]),
                            in1=dlt[:, rel:rel + nbw]
                                .rearrange("p (n f) -> p n f", f=1)
                                .to_broadcast([128, nbw, D]),
                            op=mybir.AluOpType.is_equal)
                        ps = psW.tile([128, D], f32, tag="win", space="PSUM")
                        ops = []
                        for c in range(NCH):
                            bco = int(pad[[w2 for w2 in ws if w2 < w], c]
                                      .sum()) // 128 if ws else 0
                            for j in range(int(nblk_wc[w, c])):
                                ops.append((c, bco + j,
                                            int(cblk[w, c]) + j))
                        for k, (c, b, scol) in enumerate(ops):
                            nc.tensor.matmul(
                                ps[:], lhsT=msgs[c][:, b, :],
                                rhs=Sw[:, scol, :],
                                start=(k == 0), stop=(k == len(ops) - 1))
                        nc.vector.tensor_tensor(
                            out=aggT[:, wi * 128:(wi + 1) * 128],
                            in0=ps[:], in1=dvsb[:, wi * 128:(wi + 1) * 128],
                            op=mybir.AluOpType.mult)

                    gps = psG.tile([fo, ncols], f32, tag="gps", space="PSUM")
                    nc.tensor.matmul(gps[:], lhsT=W_t[li][:, :fo],
                                     rhs=aggT[:, :ncols],
                                     start=True, stop=True)
                    cc = w0 * 128
                    if li < 2:
                        nc.scalar.copy(conv[:fo, cc:cc + ncols], gps[:])
                        sqs = sqp.tile([128, SBW * 128], f32, tag="sqs")
                        nc.scalar.square(sqs[:, :ncols],
                                         conv[:D, cc:cc + ncols])
                        nc.vector.tensor_reduce(
                            qcols[:, sbi:sbi + 1], sqs[:, :ncols],
                            mybir.AxisListType.X, mybir.AluOpType.add)
                        nc.vector.tensor_reduce(
                            scols[:, sbi:sbi + 1], conv[:D, cc:cc + ncols],
                            mybir.AxisListType.X, mybir.AluOpType.add)
                    else:
                        nc.scalar.activation(
                            conv[:fo, cc:cc + ncols], gps[:],
                            mybir.ActivationFunctionType.Identity,
                            bias=b3_t[:, 0:1], scale=1.0)

                if li == 0 and ph < 2:
                    break
                if li < 2:
                    # ---- BatchNorm global stats (accumulated per-sb) ----
                    stats = smlp.tile([128, 2], f32, tag="stats")
                    nc.vector.tensor_reduce(stats[:, 0:1], scols[:],
                                            mybir.AxisListType.X,
                                            mybir.AluOpType.add)
                    nc.vector.tensor_reduce(stats[:, 1:2], qcols[:],
                                            mybir.AxisListType.X,
                                            mybir.AluOpType.add)
                    if ph == 20:
                        break
                    ari, aro = ar_b[li]
                    nc.sync.dma_start(ari[:], stats[:])
                    if not nocc:
                        nc.gpsimd.collective_compute(
                            "AllReduce", mybir.AluOpType.add,
                            replica_groups=rg, ins=[ari.opt()],
                            outs=[aro.opt()])
                    sg = smlp.tile([128, 2], f32, tag="sg")
                    nc.sync.dma_start(sg[:], aro[:])
                    if ph == 21:
                        break
                    mean = smlp.tile([128, 1], f32, tag="mean")
                    nc.vector.tensor_scalar(mean[:], sg[:, 0:1], 1.0 / N,
                                            None, mybir.AluOpType.mult)
                    ex2 = smlp.tile([128, 1], f32, tag="ex2")
                    nc.vector.tensor_scalar(ex2[:], sg[:, 1:2], 1.0 / N,
                                            None, mybir.AluOpType.mult)
                    var = smlp.tile([128, 1], f32, tag="var")
                    nc.vector.tensor_tensor(var[:], mean[:], mean[:],
                                            op=mybir.AluOpType.mult)
                    nc.vector.tensor_tensor(var[:], ex2[:], var[:],
                                            op=mybir.AluOpType.subtract)
                    nc.vector.tensor_scalar(var[:], var[:], EPS, None,
                                            mybir.AluOpType.add)
                    std = smlp.tile([128, 1], f32, tag="std")
                    nc.scalar.sqrt(std[:], var[:])
                    istd = smlp.tile([128, 1], f32, tag="istd")
                    nc.vector.reciprocal(istd[:], std[:])
                    sco = smlp.tile([128, 1], f32, tag="sco")
                    nc.vector.tensor_tensor(sco[:], gam_t[li][:], istd[:],
                                            op=mybir.AluOpType.mult)
                    sh = smlp.tile([128, 1], f32, tag="sh")
                    nc.vector.tensor_tensor(sh[:], mean[:], sco[:],
                                            op=mybir.AluOpType.mult)
                    nc.vector.tensor_tensor(sh[:], bet_t[li][:], sh[:],
                                            op=mybir.AluOpType.subtract)
                    nc.scalar.activation(conv[:D, :NWP], conv[:D, :NWP],
                                         mybir.ActivationFunctionType.Relu,
                                         bias=sh[:, 0:1], scale=sco[:, 0:1])
                    if ph == 22:
                        break
                    prep_table(li + 1, conv)
                elif ph >= 5:
                    # ---- global mean pool + sigmoid ----
                    pooled = psP.tile([G, DOUT], f32, tag="pooled",
                                      space="PSUM")
                    for w in range(NW):
                        Gw = gwp.tile([128, G], f32, tag="Gw")
                        nc.vector.tensor_tensor(
                            out=Gw[:], in0=iota_t[:, :G],
                            in1=bat_t[:, w:w + 1].to_broadcast([128, G]),
                            op=mybir.AluOpType.is_equal)
                        t3 = psW.tile([128, D], f32, tag="win",
                                      space="PSUM")
                        nc.tensor.transpose(
                            t3[:, :DOUT], conv[:DOUT, w * 128:(w + 1) * 128],
                            id_t[:DOUT, :DOUT])
                        c3 = gwp.tile([128, DOUT], f32, tag="c3")
                        nc.scalar.copy(c3[:], t3[:, :DOUT])
                        nc.tensor.matmul(pooled[:], lhsT=Gw[:], rhs=c3[:],
                                         start=(w == 0), stop=(w == NW - 1))
                    psb = smlp.tile([G, DOUT], f32, tag="psb")
                    nc.scalar.copy(psb[:], pooled[:])
                    nc.sync.dma_start(arp_i[:], psb[:])
                    if not nocc:
                        nc.gpsimd.collective_compute(
                            "AllReduce", mybir.AluOpType.add,
                            replica_groups=rg, ins=[arp_i.opt()],
                            outs=[arp_o.opt()])
                    pall = smlp.tile([G, DOUT], f32, tag="pall")
                    nc.sync.dma_start(pall[:], arp_o[:])
                    fin = smlp.tile([G, DOUT], f32, tag="fin")
                    nc.scalar.activation(
                        fin[:], pall[:],
                        mybir.ActivationFunctionType.Sigmoid,
                        bias=0.0, scale=ci_t[:, 0:1])
                    nc.sync.dma_start(out_d[:], fin[:])

    nc.compile()
    return nc


def prepare(x, edge_index, batch, W1, b1, W2, b2, W3, b3,
            gamma1, beta1, gamma2, beta2):
    """Build the Bass program + per-core input maps."""
    layout, per_core = _prep(np.asarray(x, np.float32), edge_index, batch)
    nc = _build(layout)

    iota = np.broadcast_to(np.arange(D, dtype=np.float32), (128, D)).copy()
    ident = np.eye(D, dtype=np.float32)
    shared = {
        "iota": iota, "ident": ident,
        "W1": np.asarray(W1, np.float32), "W2": np.asarray(W2, np.float32),
        "W3": np.asarray(W3, np.float32),
        "b3": np.asarray(b3, np.float32).reshape(DOUT, 1),
        "gamma1": np.asarray(gamma1, np.float32).reshape(D, 1),
        "gamma2": np.asarray(gamma2, np.float32).reshape(D, 1),
        "beta1": np.asarray(beta1, np.float32).reshape(D, 1),
        "beta2": np.asarray(beta2, np.float32).reshape(D, 1),
    }
    in_maps = []
    for r in range(NCORES):
        pc = per_core[r]
        in_maps.append({
            "x_local": pc["x_local"], "idx16": pc["idx16"],
            "dstloc": pc["dstloc"], "dinv_local": pc["dinv_local"],
            "dinv_row": pc["dinv_row"], "batch_local": pc["batch_local"],
            "cnt_inv": pc["cnt_inv"], **shared,
        })

    return nc, in_maps


def run_on_hw(nc, in_maps):
    from concourse.bass_utils import run_bass_kernel_spmd
    last = None
    for attempt in range(3):
        try:
            res = run_bass_kernel_spmd(nc, in_maps,
                                       core_ids=list(range(NCORES)))
            return np.asarray(res.results[0]["out"], np.float32)
        except Exception as e:  # transient device wedges happen
            last = e
    raise last


def kernel(x, edge_index, batch, W1, b1, W2, b2, W3, b3,
           gamma1, beta1, gamma2, beta2):
    nc, in_maps = prepare(x, edge_index, batch, W1, b1, W2, b2, W3, b3,
                          gamma1, beta1, gamma2, beta2)
    return run_on_hw(nc, in_maps)


if __name__ == "__main__":
    sys.path.insert(0, "/root/problem")
    import reference
    inputs = {k: np.asarray(v) for k, v in reference.setup_inputs().items()}
    out = kernel(**inputs)
    print("out", out.shape, out.dtype)


# revision 24
# speedup vs baseline: 1.6068x; 1.1376x over previous
"""GCN (3-layer GCNConv + BN/ReLU + global mean pool + sigmoid) on 8 trn2
NeuronCores via Bass/Tile.

Strategy (per sharding hint): 1D-partition the 100K nodes across 8 cores
(12500 each).  Edges (incl. self-loops) are bucketed by destination core /
128-node destination window / 25000-row source chunk on the host.  Each
layer: aggregate-first formulation  conv = diag(dinv) @ A_raw @ (diag(dinv)
@ h) @ W  computed as
  - dma_gather of scaled source rows h'[src] from a replicated (AllGather'd)
    node-major table in HBM; the 4 source chunks are issued on 4 SWDGE
    queues so descriptor generation runs on 4 Q7 cpu pairs in parallel,
  - segment-sum via TensorE matmuls against one-hot indicator matrices built
    on VectorE with a broadcast is_equal against an iota row,
  - per-dst dinv scaling (rank-1 PE broadcast of the dinv row),
  - dense GEMM with the replicated [d,d] weight,
  - BatchNorm with global stats via a tiny AllReduce, fused ReLU on ScalarE.
Graph mean-pool = indicator matmul against one-hot graph ids + AllReduce.
"""
import sys
sys.path.insert(0, "/opt/trn_rl_repo")

import numpy as np

N = 100000
E = 1600000
NCORES = 8
NLOC = N // NCORES          # 12500 nodes per core
D = 128
DOUT = 32
G = 64
NW = (NLOC + 127) // 128    # 98 windows (last has 84 nodes)
NWP = NW * 128              # 12544 padded local node slots
CH = 25000                  # source chunk rows (int16-indexable)
NCH = 4
SBW = 3                     # windows per superblock
EPS = 1e-5


def _ceil128(x):
    return (np.asarray(x) + 127) // 128 * 128


def _prep(x, edge_index, batch):
    """Host-side graph partitioning. Returns (layout, per_core_arrays)."""
    src0 = np.asarray(edge_index[0], dtype=np.int64)
    dst0 = np.asarray(edge_index[1], dtype=np.int64)
    loop = np.arange(N, dtype=np.int64)
    srcs = np.concatenate([src0, loop])
    dsts = np.concatenate([dst0, loop])

    deg = np.bincount(dsts, minlength=N).astype(np.float64)
    dinv = (1.0 / np.sqrt(np.maximum(deg, 1.0))).astype(np.float32)
    dinv[deg == 0] = 0.0

    core = dsts // NLOC
    nloc = dsts % NLOC
    win = nloc >> 7
    dl = (nloc & 127).astype(np.float32)
    QQ = NLOC // 4                       # 3125 rows per core per quarter
    r_src = srcs // NLOC
    j_src = srcs % NLOC
    q_src = j_src // QQ                      # quarter 0..3 = chunk id
    rowh = r_src * QQ + (j_src - q_src * QQ)  # row within quarter table
    ch = q_src
    il = rowh.astype(np.int16)               # < 25000, int16-safe

    key = ((core * NW + win) * NCH + ch).astype(np.int64)
    order = np.argsort(key, kind="stable")
    il_s = il[order]
    dl_s = dl[order]
    cnts = np.bincount(key, minlength=NCORES * NW * NCH).reshape(
        NCORES, NW, NCH)
    starts = np.zeros(NCORES * NW * NCH + 1, np.int64)
    np.cumsum(cnts.ravel(), out=starts[1:])

    pad = _ceil128(cnts.max(axis=0)).astype(np.int64)   # [NW, NCH]
    nblk_wc = pad // 128                                # [NW, NCH]
    nblk_w = nblk_wc.sum(axis=1)                        # [NW]
    cblk = np.zeros((NW, NCH), np.int64)                # block off within win
    cblk[:, 1:] = np.cumsum(nblk_wc[:, :-1], axis=1)
    blkoff = np.zeros(NW + 1, np.int64)                 # global dstloc col off
    np.cumsum(nblk_w, out=blkoff[1:])
    nblk_tot = int(blkoff[-1])

    sbs = [list(range(i, min(i + SBW, NW))) for i in range(0, NW, SBW)]
    # idx16 column layout: per sb, per chunk call
    o16 = {}
    col16 = 0
    for sbi, ws in enumerate(sbs):
        for c in range(NCH):
            L = int(pad[ws, c].sum())
            o16[(sbi, c)] = (col16, L)
            col16 += L // 16
    cols16_tot = col16

    layout = dict(pad=pad, nblk_wc=nblk_wc, nblk_w=nblk_w, cblk=cblk,
                  blkoff=blkoff, nblk_tot=nblk_tot, sbs=sbs, o16=o16,
                  cols16_tot=cols16_tot)

    per_core = []
    batch = np.asarray(batch, dtype=np.int64)
    cnt_g = np.bincount(batch, minlength=G).astype(np.float32)
    cnt_inv = (1.0 / np.maximum(cnt_g, 1.0)).reshape(G, 1).astype(np.float32)

    for r in range(NCORES):
        idx16 = np.zeros((16, cols16_tot), np.int16)
        dstloc = np.full((128, nblk_tot), -1.0, np.float32)
        for sbi, ws in enumerate(sbs):
            for c in range(NCH):
                c0, L = o16[(sbi, c)]
                if L == 0:
                    continue
                flat = np.zeros(L, np.int16)
                q0 = 0
                for w in ws:
                    gk = (r * NW + w) * NCH + c
                    s = int(starts[gk])
                    n = int(cnts[r, w, c])
                    if n:
                        flat[q0:q0 + n] = il_s[s:s + n]
                        t = np.arange(n)
                        dstloc[t & 127,
                               blkoff[w] + cblk[w, c] + (t >> 7)] = \
                            dl_s[s:s + n]
                    q0 += int(pad[w, c])
                idx16[:, c0:c0 + L // 16] = flat.reshape(L // 16, 16).T
        idx16 = np.tile(idx16, (8, 1))  # [128, cols16_tot]

        nds = np.arange(NWP)
        gl = r * NLOC + nds
        valid = nds < NLOC
        dv = np.where(valid, dinv[np.minimum(gl, N - 1)], 0.0).astype(
            np.float32)
        dinv_local = dv.reshape(NW, 128).T.copy()          # [128, NW]
        dinv_row = dv.reshape(1, NWP).copy()               # [1, NWP]
        bl = np.where(valid, batch[np.minimum(gl, N - 1)], -1.0).astype(
            np.float32)
        batch_local = bl.reshape(NW, 128).T.copy()         # [128, NW]
        x_local = np.asarray(x[r * NLOC:(r + 1) * NLOC], np.float32)

        per_core.append(dict(idx16=idx16, dstloc=dstloc,
                             dinv_local=dinv_local, dinv_row=dinv_row,
                             batch_local=batch_local, x_local=x_local,
                             cnt_inv=cnt_inv))
    return layout, per_core


def _build(layout):
    import os
    import concourse.tile as tile
    from concourse import bacc, mybir

    ph = int(os.environ.get("KPHASE", "99"))
    nocc = bool(int(os.environ.get("KNOCC", "0")))
    kskip = os.environ.get("KSKIP", "")

    f32 = mybir.dt.float32
    DT = f32
    pad = layout["pad"]
    nblk_wc = layout["nblk_wc"]
    nblk_w = layout["nblk_w"]
    cblk = layout["cblk"]
    blkoff = layout["blkoff"]
    nblk_tot = layout["nblk_tot"]
    sbs = layout["sbs"]
    o16 = layout["o16"]
    cols16_tot = layout["cols16_tot"]

    nc = bacc.Bacc("TRN2", target_bir_lowering=False, debug=False,
                   num_devices=NCORES, num_swdge_queues=4)

    def din(name, shape, dt=f32):
        return nc.dram_tensor(name, shape, dt, kind="ExternalInput")

    x_local = din("x_local", [NLOC, D])
    idx16 = din("idx16", [128, cols16_tot], mybir.dt.int16)
    dstloc_d = din("dstloc", [128, nblk_tot])
    dinv_local_d = din("dinv_local", [128, NW])
    dinv_row_d = din("dinv_row", [1, NWP])
    batch_local_d = din("batch_local", [128, NW])
    cnt_inv_d = din("cnt_inv", [G, 1])
    iota_d = din("iota", [128, D])
    ident_d = din("ident", [128, D])
    W_d = [din("W1", [D, D]), din("W2", [D, D]), din("W3", [D, DOUT])]
    b3_d = din("b3", [DOUT, 1])
    gam_d = [din("gamma1", [D, 1]), din("gamma2", [D, 1])]
    bet_d = [din("beta1", [D, 1]), din("beta2", [D, 1])]
    out_d = nc.dram_tensor("out", [G, DOUT], f32, kind="ExternalOutput")

    from contextlib import ExitStack
    with tile.TileContext(nc) as tc, ExitStack() as _ctx:
        ec = _ctx.enter_context
        cp = ec(tc.tile_pool(name="const", bufs=1))
        convp = ec(tc.tile_pool(name="conv", bufs=1))
        xpp = ec(tc.tile_pool(name="xprep", bufs=4))
        idxp = ec(tc.tile_pool(name="idxs", bufs=3))
        dlp = ec(tc.tile_pool(name="dls", bufs=3))
        msgp = ec(tc.tile_pool(name="msg", bufs=10))
        Sp = ec(tc.tile_pool(name="Sp", bufs=2))
        aggp = ec(tc.tile_pool(name="agg", bufs=2))
        dvp = ec(tc.tile_pool(name="dv", bufs=2))
        smlp = ec(tc.tile_pool(name="sml", bufs=16))
        sqp = ec(tc.tile_pool(name="sq", bufs=2))
        gwp = ec(tc.tile_pool(name="gw", bufs=4))
        dramp = ec(tc.tile_pool(name="dram", bufs=1, space="DRAM"))
        # PSUM bank budget (8 banks): win/tp/t3 share 4 rotating slots,
        # dv 1, gemm 2, pooled 1.
        psW = ec(tc.tile_pool(name="psW", bufs=3, space="PSUM"))
        psG = ec(tc.tile_pool(name="psG", bufs=2, space="PSUM"))
        psP = ec(tc.tile_pool(name="psP", bufs=1, space="PSUM"))
        if True:
            # ---- constants ----
            iota_t = cp.tile([128, D], f32, tag="iota")
            nc.sync.dma_start(iota_t[:], iota_d[:])
            id_t = cp.tile([128, D], f32, tag="ident")
            nc.sync.dma_start(id_t[:], ident_d[:])
            dvl_t = cp.tile([128, NW], f32, tag="dvl")
            nc.sync.dma_start(dvl_t[:], dinv_local_d[:])
            bat_t = cp.tile([128, NW], f32, tag="bat")
            nc.sync.dma_start(bat_t[:], batch_local_d[:])
            ci_t = cp.tile([G, 1], f32, tag="ci")
            nc.sync.dma_start(ci_t[:], cnt_inv_d[:])
            W_t = []
            for li in range(3):
                fo = DOUT if li == 2 else D
                wt = cp.tile([D, fo], f32, tag=f"W{li}", name=f"Wt{li}")
                nc.sync.dma_start(wt[:], W_d[li][:])
                W_t.append(wt)
            b3_t = cp.tile([DOUT, 1], f32, tag="b3")
            nc.sync.dma_start(b3_t[:], b3_d[:])
            gam_t, bet_t = [], []
            for li in range(2):
                g = cp.tile([D, 1], f32, tag=f"g{li}", name=f"gam{li}")
                nc.sync.dma_start(g[:], gam_d[li][:])
                gam_t.append(g)
                b = cp.tile([D, 1], f32, tag=f"be{li}", name=f"bet{li}")
                nc.sync.dma_start(b[:], bet_d[li][:])
                bet_t.append(b)

            # ---- DRAM internals ----
            tables = [[dramp.tile([N // 4, D], DT, tag=f"table{i}{h}",
                                  name=f"table{i}{h}", addr_space="Shared")
                       for h in range(4)] for i in range(3)]
            QQ = NLOC // 4
            agins = [dramp.tile([QQ, D], DT, tag=f"agin{h}",
                                name=f"agin{h}") for h in range(4)]
            ar_b = [(dramp.tile([128, 2], f32, tag=f"ari{i}",
                                name=f"ari{i}"),
                     dramp.tile([128, 2], f32, tag=f"aro{i}",
                                name=f"aro{i}", addr_space="Shared"))
                    for i in range(2)]
            arp_i = dramp.tile([G, DOUT], f32, tag="arpi")
            arp_o = dramp.tile([G, DOUT], f32, tag="arpo",
                               addr_space="Shared")

            rg = [list(range(NCORES))]

            def prep_table(li, src_feat_major=None):
                """Write dinv-scaled node-major rows into the two agin
                quarters; AllGather each quarter as soon as it completes
                so later AGs (and the first gathers) overlap the rest."""
                fired = [False] * 4
                for w in range(NW):
                    cnt = min(128, NLOC - w * 128)
                    if src_feat_major is None:
                        xt = xpp.tile([128, D], f32, tag="xt")
                        nc.sync.dma_start(xt[:cnt, :],
                                          x_local[w * 128:w * 128 + cnt, :])
                        src_nm = xt
                    else:
                        tps = psW.tile([128, D], f32, tag="win",
                                       space="PSUM")
                        nc.tensor.transpose(
                            tps[:], src_feat_major[:D, w * 128:(w + 1) * 128],
                            id_t[:])
                        src_nm = tps
                    hq = xpp.tile([128, D], DT, tag="hq")
                    nc.scalar.activation(
                        hq[:cnt, :], src_nm[:cnt, :],
                        mybir.ActivationFunctionType.Copy,
                        bias=0.0, scale=dvl_t[:cnt, w:w + 1])
                    a = w * 128
                    b = a + cnt
                    for q in range(a // QQ, (b - 1) // QQ + 1):
                        s0 = max(a, q * QQ)
                        s1 = min(b, (q + 1) * QQ)
                        if s1 > s0:
                            nc.sync.dma_start(
                                agins[q][s0 - q * QQ:s1 - q * QQ, :],
                                hq[s0 - a:s1 - a, :])
                    for q in range(4):
                        if not fired[q] and b >= (q + 1) * QQ and not nocc:
                            fired[q] = True
                            nc.gpsimd.collective_compute(
                                "AllGather", mybir.AluOpType.bypass,
                                replica_groups=rg, ins=[agins[q].opt()],
                                outs=[tables[li][q].opt()])

            prep_table(0, None)

            conv = None
            _lireq = {0: 1, 1: 3, 2: 4}
            for li in range(3):
                if ph < _lireq[li]:
                    break
                fo = DOUT if li == 2 else D
                conv = convp.tile([128, NWP], f32, tag="conv")
                if li < 2:
                    scols = smlp.tile([128, len(sbs)], f32, tag=f"scol{li}",
                                      name=f"scol{li}", bufs=1)
                    qcols = smlp.tile([128, len(sbs)], f32, tag=f"qcol{li}",
                                      name=f"qcol{li}", bufs=1)
                for sbi, ws in enumerate(sbs):
                    ncols = len(ws) * 128
                    w0 = ws[0]
                    # stream idx + dstloc for this superblock
                    c16_0 = o16[(sbi, 0)][0]
                    c16_end = o16[(sbi, NCH - 1)][0] + \
                        o16[(sbi, NCH - 1)][1] // 16
                    idxt = idxp.tile([128, c16_end - c16_0], mybir.dt.int16,
                                     tag="idxt")
                    nc.sync.dma_start(idxt[:], idx16[:, c16_0:c16_end])
                    nb0 = int(blkoff[w0])
                    nb_sb = int(blkoff[ws[-1] + 1] - nb0)
                    dlt = dlp.tile([128, nb_sb], f32, tag="dlt")
                    nc.sync.dma_start(dlt[:], dstloc_d[:, nb0:nb0 + nb_sb])

                    msgs = []
                    for c in range(NCH):
                        c0, L = o16[(sbi, c)]
                        nblk_sc = L // 128
                        mt = msgp.tile([128, max(nblk_sc, 1), D], DT,
                                       tag="mt")
                        if L and "gather" not in kskip:
                            nc.gpsimd.dma_gather(
                                mt[:, :nblk_sc, :],
                                tables[li][c][0:CH, :],
                                idxt[:, c0 - c16_0:c0 - c16_0 + L // 16],
                                L, L, D, single_packet=False,
                                queue_num=c)
                        msgs.append(mt)

                    # dinv_dst broadcast [128, ncols]: partition-bcast DMA
                    dvsb = dvp.tile([128, ncols], f32, tag="dvsb")
                    nc.sync.dma_start(
                        dvsb[:],
                        dinv_row_d[0:1, w0 * 128:w0 * 128 + ncols]
                        .to_broadcast([128, ncols]))

                    aggT = aggp.tile([128, ncols], f32, tag="aggT")
                    for wi, w in enumerate(ws):
                        nbw = int(nblk_w[w])
                        if nbw == 0:
                            nc.vector.memset(aggT[:, wi * 128:(wi + 1) * 128],
                                             0.0)
                            continue
                        if "mm" in kskip:
                            nc.vector.memset(
                                aggT[:, wi * 128:(wi + 1) * 128], 0.0)
                            continue
                        rel = int(blkoff[w]) - nb0
                        Sw = Sp.tile([128, nbw, D], DT, tag="Sw")
                        nc.vector.tensor_tensor(
                            out=Sw[:],
                            in0=iota_t[:].rearrange("p (n f) -> p n f", n=1)
                                         .to_broadcast([128, nbw, D]),
                            in1=dlt[:, rel:rel + nbw]
                                .rearrange("p (n f) -> p n f", f=1)
                                .to_broadcast([128, nbw, D]),
                            op=mybir.AluOpType.is_equal)
                        ps = psW.tile([128, D], f32, tag="win", space="PSUM")
                        ops = []
                        for c in range(NCH):
                            bco = int(pad[[w2 for w2 in ws if w2 < w], c]
                                      .sum()) // 128 if ws else 0
                            for j in range(int(nblk_wc[w, c])):
                                ops.append((c, bco + j,
                                            int(cblk[w, c]) + j))
                        for k, (c, b, scol) in enumerate(ops):
                            nc.tensor.matmul(
                                ps[:], lhsT=msgs[c][:, b, :],
                                rhs=Sw[:, scol, :],
                                start=(k == 0), stop=(k == len(ops) - 1))
                        nc.vector.tensor_tensor(
                            out=aggT[:, wi * 128:(wi + 1) * 128],
                            in0=ps[:], in1=dvsb[:, wi * 128:(wi + 1) * 128],
                            op=mybir.AluOpType.mult)

                    gps = psG.tile([fo, ncols], f32, tag="gps", space="PSUM")
                    nc.tensor.matmul(gps[:], lhsT=W_t[li][:, :fo],
                                     rhs=aggT[:, :ncols],
                                     start=True, stop=True)
                    cc = w0 * 128
                    if li < 2:
                        nc.scalar.copy(conv[:fo, cc:cc + ncols], gps[:])
                        sqs = sqp.tile([128, SBW * 128], f32, tag="sqs")
                        nc.scalar.square(sqs[:, :ncols],
                                         conv[:D, cc:cc + ncols])
                        nc.vector.tensor_reduce(
                            qcols[:, sbi:sbi + 1], sqs[:, :ncols],
                            mybir.AxisListType.X, mybir.AluOpType.add)
                        nc.vector.tensor_reduce(
                            scols[:, sbi:sbi + 1], conv[:D, cc:cc + ncols],
                            mybir.AxisListType.X, mybir.AluOpType.add)
                    else:
                        nc.scalar.activation(
                            conv[:fo, cc:cc + ncols], gps[:],
                            mybir.ActivationFunctionType.Identity,
                            bias=b3_t[:, 0:1], scale=1.0)

                if li == 0 and ph < 2:
                    break
                if li < 2:
                    # ---- BatchNorm global stats (accumulated per-sb) ----
                    stats = smlp.tile([128, 2], f32, tag="stats")
                    nc.vector.tensor_reduce(stats[:, 0:1], scols[:],
                                            mybir.AxisListType.X,
                                            mybir.AluOpType.add)
                    nc.vector.tensor_reduce(stats[:, 1:2], qcols[:],
                                            mybir.AxisListType.X,
                                            mybir.AluOpType.add)
                    if ph == 20:
                        break
                    ari, aro = ar_b[li]
                    nc.sync.dma_start(ari[:], stats[:])
                    if not nocc:
                        nc.gpsimd.collective_compute(
                            "AllReduce", mybir.AluOpType.add,
                            replica_groups=rg, ins=[ari.opt()],
                            outs=[aro.opt()])
                    sg = smlp.tile([128, 2], f32, tag="sg")
                    nc.sync.dma_start(sg[:], aro[:])
                    if ph == 21:
                        break
                    mean = smlp.tile([128, 1], f32, tag="mean")
                    nc.vector.tensor_scalar(mean[:], sg[:, 0:1], 1.0 / N,
                                            None, mybir.AluOpType.mult)
                    ex2 = smlp.tile([128, 1], f32, tag="ex2")
                    nc.vector.tensor_scalar(ex2[:], sg[:, 1:2], 1.0 / N,
                                            None, mybir.AluOpType.mult)
                    var = smlp.tile([128, 1], f32, tag="var")
                    nc.vector.tensor_tensor(var[:], mean[:], mean[:],
                                            op=mybir.AluOpType.mult)
                    nc.vector.tensor_tensor(var[:], ex2[:], var[:],
                                            op=mybir.AluOpType.subtract)
                    nc.vector.tensor_scalar(var[:], var[:], EPS, None,
                                            mybir.AluOpType.add)
                    std = smlp.tile([128, 1], f32, tag="std")
                    nc.scalar.sqrt(std[:], var[:])
                    istd = smlp.tile([128, 1], f32, tag="istd")
                    nc.vector.reciprocal(istd[:], std[:])
                    sco = smlp.tile([128, 1], f32, tag="sco")
                    nc.vector.tensor_tensor(sco[:], gam_t[li][:], istd[:],
                                            op=mybir.AluOpType.mult)
                    sh = smlp.tile([128, 1], f32, tag="sh")
                    nc.vector.tensor_tensor(sh[:], mean[:], sco[:],
                                            op=mybir.AluOpType.mult)
                    nc.vector.tensor_tensor(sh[:], bet_t[li][:], sh[:],
                                            op=mybir.AluOpType.subtract)
                    nc.scalar.activation(conv[:D, :NWP], conv[:D, :NWP],
                                         mybir.ActivationFunctionType.Relu,
                                         bias=sh[:, 0:1], scale=sco[:, 0:1])
                    if ph == 22:
                        break
                    prep_table(li + 1, conv)
                elif ph >= 5:
                    # ---- global mean pool + sigmoid ----
                    pooled = psP.tile([G, DOUT], f32, tag="pooled",
                                      space="PSUM")
                    for w in range(NW):
                        Gw = gwp.tile([128, G], f32, tag="Gw")
                        nc.vector.tensor_tensor(
                            out=Gw[:], in0=iota_t[:, :G],
                            in1=bat_t[:, w:w + 1].to_broadcast([128, G]),
                            op=mybir.AluOpType.is_equal)
                        t3 = psW.tile([128, D], f32, tag="win",
                                      space="PSUM")
                        nc.tensor.transpose(
                            t3[:, :DOUT], conv[:DOUT, w * 128:(w + 1) * 128],
                            id_t[:DOUT, :DOUT])
                        c3 = gwp.tile([128, DOUT], f32, tag="c3")
                        nc.scalar.copy(c3[:], t3[:, :DOUT])
                        nc.tensor.matmul(pooled[:], lhsT=Gw[:], rhs=c3[:],
                                         start=(w == 0), stop=(w == NW - 1))
                    psb = smlp.tile([G, DOUT], f32, tag="psb")
                    nc.scalar.copy(psb[:], pooled[:])
                    nc.sync.dma_start(arp_i[:], psb[:])
                    if not nocc:
                        nc.gpsimd.collective_compute(
                            "AllReduce", mybir.AluOpType.add,
                            replica_groups=rg, ins=[arp_i.opt()],
                            outs=[arp_o.opt()])
                    pall = smlp.tile([G, DOUT], f32, tag="pall")
                    nc.sync.dma_start(pall[:], arp_o[:])
                    fin = smlp.tile([G, DOUT], f32, tag="fin")
                    nc.scalar.activation(
                        fin[:], pall[:],
                        mybir.ActivationFunctionType.Sigmoid,
                        bias=0.0, scale=ci_t[:, 0:1])
                    nc.sync.dma_start(out_d[:], fin[:])

    nc.compile()
    return nc


def prepare(x, edge_index, batch, W1, b1, W2, b2, W3, b3,
            gamma1, beta1, gamma2, beta2):
    """Build the Bass program + per-core input maps."""
    layout, per_core = _prep(np.asarray(x, np.float32), edge_index, batch)
    nc = _build(layout)

    iota = np.broadcast_to(np.arange(D, dtype=np.float32), (128, D)).copy()
    ident = np.eye(D, dtype=np.float32)
    shared = {
        "iota": iota, "ident": ident,
        "W1": np.asarray(W1, np.float32), "W2": np.asarray(W2, np.float32),
        "W3": np.asarray(W3, np.float32),
        "b3": np.asarray(b3, np.float32).reshape(DOUT, 1),
        "gamma1": np.asarray(gamma1, np.float32).reshape(D, 1),
        "gamma2": np.asarray(gamma2, np.float32).reshape(D, 1),
        "beta1": np.asarray(beta1, np.float32).reshape(D, 1),
        "beta2": np.asarray(beta2, np.float32).reshape(D, 1),
    }
    in_maps = []
    for r in range(NCORES):
        pc = per_core[r]
        in_maps.append({
            "x_local": pc["x_local"], "idx16": pc["idx16"],
            "dstloc": pc["dstloc"], "dinv_local": pc["dinv_local"],
            "dinv_row": pc["dinv_row"], "batch_local": pc["batch_local"],
            "cnt_inv": pc["cnt_inv"], **shared,
        })

    return nc, in_maps


def run_on_hw(nc, in_maps):
    from concourse.bass_utils import run_bass_kernel_spmd
    last = None
    for attempt in range(3):
        try:
            res = run_bass_kernel_spmd(nc, in_maps,
                                       core_ids=list(range(NCORES)))
            return np.asarray(res.results[0]["out"], np.float32)
        except Exception as e:  # transient device wedges happen
            last = e
    raise last


def kernel(x, edge_index, batch, W1, b1, W2, b2, W3, b3,
           gamma1, beta1, gamma2, beta2):
    nc, in_maps = prepare(x, edge_index, batch, W1, b1, W2, b2, W3, b3,
                          gamma1, beta1, gamma2, beta2)
    return run_on_hw(nc, in_maps)


if __name__ == "__main__":
    sys.path.insert(0, "/root/problem")
    import reference
    inputs = {k: np.asarray(v) for k, v in reference.setup_inputs().items()}
    out = kernel(**inputs)
    print("out", out.shape, out.dtype)
